# revision 54
# baseline (speedup 1.0000x reference)
"""Attention2d Trainium2 Bass kernel.

Reference computation (per batch element b of 8, one NeuronCore each):
    hn  = GroupNorm32(x) * gn1_scale + gn1_bias
    qkv = w_qkv @ hn + b_qkv          (1x1 conv == matmul over channels)
    per head h (8 heads, ch=64): q,k,v from qkv (torch reshape convention:
        head h uses rows h*192+{0..64,64..128,128..192})
    wgt = softmax((q*s)^T (k*s)), s = ch**-0.25
    a   = v @ wgt^T
    out = GroupNorm32(w_proj @ a + b_proj) ... * gn2_scale + gn2_bias
    y   = x + out

Device strategy (data-parallel over batch, 1 core per batch element):
  - ACT is the critical engine: the 8 heads' exp(S^T) stream is 64 tiles of
    [128, 1024] (~1.04us each). Everything else is scheduled so that stream
    never starves: PE work is held well under the stream duration.
  - S^T runs as fp8e4 DoubleRow matmuls at 0.5 cycles/row. To keep the
    quantization error inside the rel-err budget, q ships as TWO fp8
    k-subtiles (hi + residual lo, together fp16-accurate) against a
    DUPLICATED fp8 k: S = k8^T(q_hi + q_lo), so only k's single fp8
    rounding touches the logits. q_lo is formed by accumulating -I @ q_hi
    into the conv PSUM (one cheap fp8 matmul) and re-evacuating.
  - exps, v^T tiles, the A matmuls, convs and proj all stay fp16: their
    quantization hits the output directly (measured: fp8 exps alone cost
    2.6e-2 rel err), while the k-side fp8 is dampened through softmax.
  - the A matmul runs TRANSPOSED: out a^T[t-part, ch] per 128-wide t-chunk;
    the softmax denominator (ones-column of v^T) lands in column 64 as a
    per-partition scalar: one DVE reciprocal + one fused tensor_scalar
    normalizes while evacuating. The LAST head's A accumulation is split
    into s-halves so only half of it trails the final exp tile.
  - a^T -> a via XBAR dma_start_transpose (no engine time); the LAST pair
    uses PE identity-transposes so proj isn't tail-gated by the HWDGE queue
  - v-bias folded into the proj bias on the host; proj bias folded into the
    GN2 affine + channel stats
  - GN2: DVE bn_stats reads each proj PSUM tile directly (one pass), the
    group reduce + affine run on PE/ACT (idle post-stream), and a single
    ACT Identity applies y = ps*A + B straight out of PSUM
  - input DMAs are packed and ordered so GN1 stats chase the x chunks
    (ACT takes the chunk that lands first, DVE the rest), pair-0 weights
    ship in their own small DMA, and pair-0's q/k evacuations run on the
    still-idle ACT so the first exp fires as early as possible
  - scratch warm-up matmuls on a memset tile hold the PE in its fast
    p-state from t~0
"""

import numpy as np

NHEAD = 8
GROUPS = 32
EPS = 1e-5
B, C, H, W = 8, 512, 32, 32
N = H * W            # 1024 spatial positions
CH = C // NHEAD      # 64 channels per head
P = 128              # partitions
KC = C // P          # 4 channel chunks
NT = N // 512        # 2 column tiles of 512
SC = N // P          # 8 s-chunks
GC = GROUPS // KC    # 8 groups per chunk

_CACHE = {}


def _build_nc():
    import concourse.tile as tile
    from concourse import mybir, bacc
    from concourse.hw_specs import get_activation_tables

    f32 = mybir.dt.float32
    f16 = mybir.dt.float16
    f8 = mybir.dt.float8e4
    i16 = mybir.dt.int16
    AF = mybir.ActivationFunctionType
    OP = mybir.AluOpType
    PM = mybir.MatmulPerfMode

    nc = bacc.Bacc("TRN2", target_bir_lowering=False, num_devices=8)

    WI = 2 * KC * C          # wv/wp block columns in the packed weight tile
    WT = WI + P + P // 2     # + f16 identity + fp8 -identity (bit-packed)

    x_d = nc.dram_tensor("x", [P, KC, N], f32, kind="ExternalInput")
    x8_d = nc.dram_tensor("x8", [P, KC, N], f16, kind="ExternalInput")
    wqk_d = nc.dram_tensor("wqk", [P, KC, 2, KC, P], f16, kind="ExternalInput")
    wvp_d = nc.dram_tensor("wvp", [P, WT], f16, kind="ExternalInput")
    pars_d = nc.dram_tensor("pars", [P, 448], f32, kind="ExternalInput")
    parsg_d = nc.dram_tensor("parsg", [GROUPS, 1544], f32, kind="ExternalInput")
    out_d = nc.dram_tensor("out", [P, KC, N], f32, kind="ExternalOutput")

    with tile.TileContext(nc) as tc:
        with (
            tc.tile_pool(name="big", bufs=1) as big,
            tc.tile_pool(name="wpool", bufs=2) as wpool,
            tc.tile_pool(name="qpool", bufs=2) as qpool,
            tc.tile_pool(name="kpool", bufs=4) as kpool,
            tc.tile_pool(name="vtp", bufs=1) as vtp,
            tc.tile_pool(name="expp", bufs=3) as expp,
            tc.tile_pool(name="autp", bufs=2) as autp,
            tc.tile_pool(name="tmpp", bufs=2) as tmpp,
            tc.tile_pool(name="stp", bufs=2, space="PSUM") as stp,
            tc.tile_pool(name="apool", bufs=2, space="PSUM") as apool,
            tc.tile_pool(name="qp", bufs=2, space="PSUM") as qp,
        ):
            # ---------- persistent SBUF tiles ----------
            x_sb = big.tile([P, KC, N], f16, tag="x_sb")
            hn = big.tile([P, KC, N], f16, tag="hn")
            vt = vtp.tile([P, SC, NHEAD, CH + 1], f16, tag="vp")
            pars_sb = big.tile([P, 448], f32, tag="pars_sb")
            parsg_sb = big.tile([GROUPS, 1544], f32, tag="parsg_sb")
            ab1 = big.tile([P, KC, 2], f32, tag="ab1")
            ab2 = big.tile([P, KC, 2], f32, tag="ab2")
            projf = big.tile([P, KC, N], f32, tag="projf")
            a_u = big.tile([P, KC, N], f16, tag="a_u")
            scr16 = big.tile([P, 2, N], f16, tag="scr16")
            wmup = big.tile([P, 512], f16, tag="wmup")

            # packed-parameter views
            g1s = pars_sb[:, 0:4]
            g1b = pars_sb[:, 4:8]
            bq_sb = pars_sb[:, 8:12]
            bk_sb = pars_sb[:, 12:16]
            bp_sb = pars_sb[:, 16:20]
            g2s = pars_sb[:, 20:24]
            g2b = pars_sb[:, 24:28]
            sel8 = pars_sb[:, 28:60].rearrange("p (k g) -> p k g", k=KC)
            sel32 = pars_sb[:, 60:188].rearrange("p (k g) -> p k g", k=KC)
            onescol = pars_sb[:, 444:445]
            selt32 = parsg_sb[:, 0:512].rearrange("g (k c) -> g k c", k=KC)
            selt8 = parsg_sb[0:GC, 1028:1540].rearrange("g (k c) -> g k c", k=KC)
            eps32 = parsg_sb[:, 1024:1025]
            eps8 = parsg_sb[0:GC, 1024:1025]

            # PE warm-up from t~0: matmuls on a Pool-memset scratch tile hold
            # the PE through its p-state ramp so the first real matmuls run at
            # full speed. Results are discarded.
            nc.gpsimd.memset(wmup[:], 0.0)
            # enough back-to-back warm-up matmuls to bridge to the first GN1
            # group matmuls (~8us) -- a >~2us PE idle gap resets the ramp
            for _ in range(26):
                ps_w = qp.tile([P, 512], f32, tag="qp")
                nc.tensor.matmul(
                    ps_w[:, 0:448],
                    wmup[:, 0:128],
                    wmup[:, 0:448],
                    start=True, stop=True,
                )

            # ---------- input DMAs, ordered for the GN1 -> conv chain -------
            # chunk 2 lands first (its stats run on ACT), then chunk 3 so the
            # DVE's last bn_stats isn't the straggler; pair-0 weights ship in
            # their own small contiguous DMA.
            nc.sync.dma_start(x_sb[:, 2, :], x8_d[:, 2, :])
            nc.sync.dma_start(x_sb[:, 3, :], x8_d[:, 3, :])
            nc.sync.dma_start(x_sb[:, 0, :], x8_d[:, 0, :])
            nc.sync.dma_start(x_sb[:, 1, :], x8_d[:, 1, :])
            nc.sync.dma_start(pars_sb[:], pars_d[:])
            nc.sync.dma_start(parsg_sb[:], parsg_d[:])
            wqk_sb = wpool.tile([P, KC, 2, KC, P], f16, tag="wqk")
            nc.sync.dma_start(wqk_sb[:, 0], wqk_d[:, 0])
            nc.sync.dma_start(wqk_sb[:, 1:KC], wqk_d[:, 1:KC])
            wvp_sb = wpool.tile([P, WT], f16, tag="wvp")
            nc.sync.dma_start(wvp_sb[:], wvp_d[:])
            nc.sync.dma_start(out_d[:], x_d[:])

            wq_sb = wqk_sb[:, :, 0]
            wk_sb = wqk_sb[:, :, 1]
            wv_sb = wvp_sb[:, 0 : KC * C].rearrange("p (k c) -> p k c", k=KC)
            wp_sb = wvp_sb[:, KC * C : WI].rearrange("p (k c) -> p k c", k=KC)
            ident_sb = wvp_sb[:, WI : WI + P]
            negid8 = wvp_sb[:, WI + P : WT].bitcast(f8)

            # Preload the combined ln+exp ACT table set once (Ln/Exp are used
            # for the GroupNorm rstd), so the bacc table-load pass doesn't
            # thrash between sets.
            _set_names = list(get_activation_tables(nc.m.arch).keys())
            _tl = mybir.InstLoadActFuncSet(
                name=nc.get_next_instruction_name(),
                ins=[],
                outs=[],
                act_func_set_id=_set_names.index("natural_log_exp_and_others"),
            )
            _tl.engine = mybir.EngineType.Activation
            nc.scalar.add_instruction(_tl)

            nc.gpsimd.tensor_copy(
                out=vt[:, :, :, CH : CH + 1],
                in_=onescol[:, :, None, None].to_broadcast((P, SC, NHEAD, 1)),
            )

            # ---------- per-chunk GroupNorm chain (used by GN2) ----------
            def gn_chunk(k, mvs_ap, gs, gb, ab, bias_fold):
                """ab[:, k] = per-channel (A, B) for y = src*A + B, given
                per-channel (mean, var) in mvs_ap ([P, 2], chunk k); the
                [P, 1] AP bias_fold adjusts the stats and B as if it had been
                added to the source. DVE ops read the group-reduce PSUM
                directly to keep the dependency chain short."""
                stat2 = tmpp.tile([P, 2], f32, tag="stat2")
                nc.vector.tensor_tensor(
                    stat2[:, 0:1], mvs_ap[:, 0:1], bias_fold, OP.add
                )
                musq = tmpp.tile([P, 1], f32, tag="musq")
                nc.vector.tensor_tensor(musq[:], stat2[:, 0:1], stat2[:, 0:1], OP.mult)
                nc.vector.tensor_tensor(stat2[:, 1:2], mvs_ap[:, 1:2], musq[:], OP.add)
                ps_g = apool.tile([P, 260], f32, tag="apool")
                nc.tensor.matmul(
                    ps_g[0:GC, 0:2], sel8[:, k, :], stat2[:, :],
                    start=True, stop=True,
                )
                gstat = tmpp.tile([GC, 2], f32, tag="gstat")
                nc.vector.tensor_copy(out=gstat[:, 0:1], in_=ps_g[0:GC, 0:1])
                gvar = tmpp.tile([GC, 1], f32, tag="gvar")
                gmusq = tmpp.tile([GC, 1], f32, tag="gmusq")
                # only one PSUM operand allowed per instruction: square the
                # SBUF copy of the group mean
                nc.vector.tensor_tensor(gmusq[:], gstat[:, 0:1], gstat[:, 0:1], OP.mult)
                nc.vector.tensor_tensor(gvar[:], ps_g[0:GC, 1:2], gmusq[:], OP.subtract)
                nc.scalar.activation(out=gvar[:], in_=gvar[:], func=AF.Ln, bias=eps8)
                nc.scalar.activation(out=gstat[:, 1:2], in_=gvar[:], func=AF.Exp, scale=-0.5)
                ps_c = apool.tile([P, 260], f32, tag="apool")
                nc.tensor.matmul(
                    ps_c[:, 0:2], selt8[:, k, :], gstat[:],
                    start=True, stop=True,
                )
                nc.vector.tensor_tensor(
                    ab[:, k, 0:1], gs[:, k : k + 1], ps_c[:, 1:2], OP.mult
                )
                # B = gb + A*(bias_fold - mean_c)
                ma = tmpp.tile([P, 2], f32, tag="ma")
                nc.vector.tensor_tensor(ma[:, 0:1], bias_fold, ps_c[:, 0:1], OP.subtract)
                nc.vector.tensor_tensor(ma[:, 1:2], ab[:, k, 0:1], ma[:, 0:1], OP.mult)
                nc.vector.tensor_tensor(
                    ab[:, k, 1:2], gb[:, k : k + 1], ma[:, 1:2], OP.add
                )

            # ---------- GN1 -> hn (stats chase the x chunk DMAs: ACT
            # accumulators for chunk 2 (lands first), DVE bn_stats for chunks
            # 3,0,1 in arrival order; one batched group reduce + affine) -----
            BN_CHUNKS = (3, 0, 1)
            mvs1 = big.tile([P, 3, 2], f32, tag="mvs1")
            stat2 = big.tile([P, KC, 2], f32, tag="stat2b")
            nc.scalar.activation(
                out=scr16[:, 0, :], in_=x_sb[:, 2, :], func=AF.Copy,
                scale=1.0 / N, accum_out=stat2[:, 2, 0:1],
            )
            nc.scalar.activation(
                out=scr16[:, 1, :], in_=x_sb[:, 2, :], func=AF.Square,
                scale=1.0 / 32.0, accum_out=stat2[:, 2, 1:2],
            )
            for i, k in enumerate(BN_CHUNKS):
                stats = tmpp.tile([P, 2, 6], f32, tag="bnstats")
                resh = x_sb[:, k, :].rearrange("p (s f) -> p s f", f=512)
                for si in range(2):
                    nc.vector.bn_stats(out=stats[:, si, :], in_=resh[:, si, :])
                nc.vector.bn_aggr(out=mvs1[:, i, :], in_=stats[:])
            musq = tmpp.tile([P, 3], f32, tag="musqb")
            nc.vector.tensor_tensor(musq[:], mvs1[:, :, 0], mvs1[:, :, 0], OP.mult)
            for i, k in enumerate(BN_CHUNKS):
                nc.vector.tensor_tensor(
                    stat2[:, k, 1:2], mvs1[:, i, 1:2], musq[:, i : i + 1], OP.add
                )
                nc.vector.tensor_copy(out=stat2[:, k, 0:1], in_=mvs1[:, i, 0:1])
            ps_g1 = qp.tile([P, 512], f32, tag="qp")
            for k in range(KC):
                nc.tensor.matmul(
                    ps_g1[0:GROUPS, 0:2], sel32[:, k, :], stat2[:, k, :],
                    start=(k == 0), stop=(k == KC - 1),
                )
            gst1 = big.tile([GROUPS, 2], f32, tag="gst1")
            gms1 = tmpp.tile([GROUPS, 2], f32, tag="gms1")
            nc.vector.tensor_copy(out=gms1[:], in_=ps_g1[0:GROUPS, 0:2])
            nc.vector.tensor_copy(out=gst1[:, 0:1], in_=gms1[:, 0:1])
            gv1 = tmpp.tile([GROUPS, 1], f32, tag="gv1")
            gmu1 = tmpp.tile([GROUPS, 1], f32, tag="gmu1")
            nc.vector.tensor_tensor(gmu1[:], gms1[:, 0:1], gms1[:, 0:1], OP.mult)
            nc.vector.tensor_tensor(gv1[:], gms1[:, 1:2], gmu1[:], OP.subtract)
            nc.scalar.activation(out=gv1[:], in_=gv1[:], func=AF.Ln, bias=eps32)
            nc.scalar.activation(out=gst1[:, 1:2], in_=gv1[:], func=AF.Exp, scale=-0.5)
            ps_c1 = qp.tile([P, 512], f32, tag="qp")
            for k in range(KC):
                nc.tensor.matmul(
                    ps_c1[:, 2 * k : 2 * k + 2], selt32[:, k, :], gst1[:],
                    start=True, stop=True,
                )
            cst1 = tmpp.tile([P, KC, 2], f32, tag="cst1")
            nc.vector.tensor_copy(out=cst1[:], in_=ps_c1[:, 0 : 2 * KC])
            nc.vector.tensor_tensor(ab1[:, :, 0], g1s[:, :], cst1[:, :, 1], OP.mult)
            ma1 = tmpp.tile([P, KC], f32, tag="ma1")
            nc.vector.tensor_tensor(ma1[:], cst1[:, :, 0], ab1[:, :, 0], OP.mult)
            nc.vector.tensor_tensor(ab1[:, :, 1], g1b[:, :], ma1[:], OP.subtract)
            for k in range(KC):
                nc.vector.tensor_scalar(
                    hn[:, k, :], x_sb[:, k, :],
                    ab1[:, k, 0:1], ab1[:, k, 1:2], OP.mult, OP.add,
                )

            # ---------- phase helpers ----------
            def evac(eng, dst, src, bias_ap):
                if eng is None:
                    # ACT bias-add copy (idle pre-stream)
                    nc.scalar.activation(
                        out=dst, in_=src, func=AF.Identity, bias=bias_ap
                    )
                else:
                    eng.tensor_scalar(dst, src, bias_ap, None, OP.add)

            def conv_q(p, qt, eng):
                # q as hi+lo fp8 pair: evac hi, subtract rne8(hi) from the
                # PSUM via an accumulated -I @ hi matmul, evac the residual
                # on the (otherwise idle) Pool so the DVE keeps up with the
                # Schraudolph share of the exp stream.
                bias_ap = bq_sb[:, p : p + 1]
                for t in range(NT):
                    ts_ = slice(t * 512, (t + 1) * 512)
                    ps = qp.tile([P, 512], f32, tag="qp")
                    for k in range(KC):
                        nc.tensor.matmul(
                            ps[:, :],
                            wq_sb[:, p, k, :],
                            hn[:, k, ts_],
                            start=(k == 0), stop=(k == KC - 1),
                        )
                    evac(eng, qt[:, 0, ts_], ps[:], bias_ap)
                    nc.tensor.matmul(
                        ps[:, :], negid8[:], qt[:, 0, ts_],
                        start=False, stop=True, skip_group_check=True,
                    )
                    # pair 0's lo goes to the DVE (free once hn is done) so
                    # the ACT-side hi evacs and the first exps aren't queued
                    # behind it
                    evac(nc.vector if eng is None else eng,
                         qt[:, 1, ts_], ps[:], bias_ap)

            def conv_k(p, kt, eng, trange=range(NT)):
                bias_ap = bk_sb[:, p : p + 1]
                for t in trange:
                    ts_ = slice(t * 512, (t + 1) * 512)
                    ps = qp.tile([P, 512], f32, tag="qp")
                    for k in range(KC):
                        nc.tensor.matmul(
                            ps[:, :],
                            wk_sb[:, p, k, :],
                            hn[:, k, ts_],
                            start=(k == 0), stop=(k == KC - 1),
                        )
                    evac(eng, kt[:, 0, ts_], ps[:], bias_ap)
                    # duplicate into subtile 1 for the DoubleRow layout
                    nc.gpsimd.tensor_copy(out=kt[:, 1, ts_], in_=kt[:, 0, ts_])

            def v_tiles(half):
                # v^T tiles [s-part, head-major channel]; bv is folded into
                # the proj bias on the host, so no bias row here. Built in
                # halves slotted into heads 0 and 1 so the DVE evacuation
                # copies don't pile up in one stream window.
                for nt in range(4 * half, 4 * half + 4):
                    ps = qp.tile([P, 512], f32, tag="qp")
                    for k in range(KC):
                        nc.tensor.matmul(
                            ps[:, :],
                            hn[:, k, nt * P : (nt + 1) * P],
                            wv_sb[:, k, :],
                            start=(k == 0), stop=(k == KC - 1),
                        )
                    nc.vector.tensor_copy(
                        out=vt[:, nt, :, 0:CH],
                        in_=ps[:, :].rearrange("p (h c) -> p h c", h=NHEAD),
                    )

            # s-chunks routed to the DVE via the Schraudolph 2^x bit trick:
            # i16 = round(S*1024*log2(e) + (15*1024 - 62.2)) reinterpreted as
            # fp16 approximates exp(S) to ~+-4% -- the softmax denominator
            # uses the same approximated values, so the common mode cancels.
            SCHRA = (2, 5)
            SC1 = 1024 * 1.4426950408889634
            SC2 = 15360.0 - 62.2

            def head_st(h, qt, kt, mid=None):
                # S^T as fp8 DoubleRow: subtiles = (q_hi, q_lo) against a
                # duplicated k8, so each [128, 512] output costs 256 PE
                # cycles at near-fp16 accuracy.
                p, e = h // 2, h % 2
                rows = slice(64 * e, 64 * e + 64)
                exps = expp.tile([P, SC, N], f16, tag="exps")
                for sc in range(SC):
                    if sc == 2 and mid is not None:
                        mid()
                    if sc in SCHRA:
                        # Schraudolph tiles go through qp halves so the
                        # ACT-fed stp pipeline never waits on the DVE
                        for t in range(NT):
                            pq = qp.tile([P, 512], f32, tag="qp")
                            nc.tensor.matmul(
                                pq[:, :],
                                kt[rows, :, sc * P : (sc + 1) * P],
                                qt[rows, :, t * 512 : (t + 1) * 512],
                                start=True, stop=True,
                                perf_mode=PM.DoubleRow,
                            )
                            nc.vector.tensor_scalar(
                                exps[:, sc, t * 512 : (t + 1) * 512].bitcast(i16),
                                pq[:], SC1, SC2, OP.mult, OP.add,
                            )
                        continue
                    ps_st = stp.tile([P, N], f32, tag="stp")
                    for t in range(NT):
                        nc.tensor.matmul(
                            ps_st[:, t * 512 : (t + 1) * 512],
                            kt[rows, :, sc * P : (sc + 1) * P],
                            qt[rows, :, t * 512 : (t + 1) * 512],
                            start=True, stop=True,
                            perf_mode=PM.DoubleRow,
                        )
                    nc.scalar.activation(
                        out=exps[:, sc, :], in_=ps_st[:], func=AF.Exp
                    )
                return exps

            def head_a(h, exps, auT):
                # Transposed A: out a^T[t-part, ch] per 128-wide t-chunk, the
                # softmax denominator lands in column 64 as a per-partition
                # scalar -> one reciprocal + a fused normalize-evacuate.
                # (NOTE: the s accumulation must NOT interleave j groups --
                # PSUM allows one pending accumulation group per bank.)
                e = h % 2
                for u in range(2):
                    ps_aT = apool.tile([P, 260], f32, tag="apool")
                    pv = ps_aT[:].rearrange("p (j c) -> p j c", c=65)
                    for j in range(4):
                        tch = 4 * u + j
                        for sc in range(SC):
                            nc.tensor.matmul(
                                pv[:, j, :],
                                exps[:, sc, tch * P : (tch + 1) * P],
                                vt[:, sc, h, :],
                                start=(sc == 0), stop=(sc == SC - 1),
                            )
                    rcol = tmpp.tile([P, 4], f32, tag="rcol")
                    nc.vector.reciprocal(rcol[:], pv[:, :, 64])
                    for j in range(4):
                        tch = 4 * u + j
                        nc.vector.tensor_scalar(
                            auT[:, tch, 64 * e : 64 * e + 64], pv[:, j, 0:64],
                            rcol[:, j : j + 1], None, OP.mult,
                        )

            # ---------- qkv, then attention ----------
            def conv_pair(p, eng):
                qt = qpool.tile([P, 2, N], f8, tag="qt")
                kt = kpool.tile([P, 2, N], f8, tag="kt")
                conv_k(p, kt, eng, trange=(0,))
                conv_q(p, qt, eng)
                conv_k(p, kt, eng, trange=(1,))
                return qt, kt

            # pair 0 evacuates on the still-idle ACT so the stream starts
            # as early as possible; later pairs use the DVE.
            qt, kt = conv_pair(0, None)
            e = {}
            e[0] = head_st(0, qt, kt, mid=lambda: v_tiles(0))
            # v^T tiles build on the PE while head 0/1's exps stream, slotted
            # into the middles of both heads' S^T so exp starts on time
            e[1] = head_st(1, qt, kt, mid=lambda: v_tiles(1))
            auTs = {}

            def transposes(p):
                # chunks 0/1 ride the XBAR mid-stream; chunks 2/3 take PE
                # identity-transposes -- the XBAR's HWDGE serialization
                # (~5us for 8 tiles + ~1us completion sem) would gate the
                # proj pre-runs right at the stream tail
                if p < KC - 2:
                    # XBAR transpose a^T -> a_u chunk p (SBUF->SBUF, no
                    # PE/DVE time)
                    for tch in range(SC):
                        nc.sync.dma_start_transpose(
                            a_u[:, p, tch * P : (tch + 1) * P], auTs[p][:, tch, :]
                        )
                else:
                    # tail chunk: PE transposes (identity matmul) -> shortest
                    # path into proj's k=3 contraction; the PSUM->SBUF copies
                    # run on the post-stream-idle ACT so the DVE (busy with
                    # the last head's normalize) isn't the serializer
                    for tch in range(SC):
                        ps_t = apool.tile([P, 260], f32, tag="apool")
                        pt = ps_t[:].bitcast(f16)
                        nc.tensor.matmul(
                            pt[:, 0:P], auTs[p][:, tch, :], ident_sb[:],
                            is_transpose=True,
                        )
                        if p == KC - 1:
                            # post-stream: ACT is idle, keep the DVE free for
                            # the bn/gn chains
                            nc.scalar.activation(
                                out=a_u[:, p, tch * P : (tch + 1) * P],
                                in_=pt[:, 0:P], func=AF.Copy,
                            )
                        else:
                            nc.vector.tensor_copy(
                                out=a_u[:, p, tch * P : (tch + 1) * P],
                                in_=pt[:, 0:P],
                            )

            # software pipeline: S^T/exp of pair p streams while the A
            # matmuls of pair p-1 drain, so pair boundaries stay dense
            for p in range(1, KC):
                qt, kt = conv_pair(p, nc.vector)
                h0 = 2 * (p - 1)
                e[2 * p] = head_st(2 * p, qt, kt)
                auT = autp.tile([P, SC, P], f16, tag="auT")
                auTs[p - 1] = auT
                head_a(h0, e[h0], auTs[p - 1])
                e[2 * p + 1] = head_st(2 * p + 1, qt, kt)
                head_a(h0 + 1, e[h0 + 1], auTs[p - 1])
                transposes(p - 1)
            auT = autp.tile([P, SC, P], f16, tag="auT")
            auTs[KC - 1] = auT
            head_a(6, e[6], auTs[KC - 1])

            head_a(7, e[7], auTs[KC - 1])

            # proj m0's k<3 accumulation also pre-runs under the stream tail
            preruns = {}
            ps_pm0 = stp.tile([P, N], f32, tag="stp")
            preruns[0] = ps_pm0
            for t in range(NT):
                for k in range(KC - 1):
                    nc.tensor.matmul(
                        preruns[0][:, t * 512 : (t + 1) * 512],
                        wp_sb[:, k, 0:P],
                        a_u[:, k, t * 512 : (t + 1) * 512],
                        start=(k == 0), stop=False,
                        skip_group_check=True,
                    )

            transposes(KC - 1)

            # ---------- proj + GN2 + residual, pipelined per chunk ----------
            # DVE bn_stats reads the proj PSUM directly (one pass for mean
            # and var); the per-chunk group reduce + affine then run on the
            # post-stream-idle PE/ACT, and one ACT Identity applies
            # y = ps*A + B straight out of PSUM. The (host-folded) proj bias
            # enters the stats and the B term via gn_chunk's bias_fold.
            mvs2 = big.tile([P, KC, 2], f32, tag="mvs2")
            # m0 and m1 finish their pre-run accumulators and take their
            # stats back-to-back BEFORE either gn chain runs, so the second
            # bn pass isn't stuck behind the first gn's PE/ACT hops in the
            # in-order DVE queue
            pr_halves = {}
            for m in preruns:
                halves = [
                    preruns[m][:, t * 512 : (t + 1) * 512] for t in range(NT)
                ]
                pr_halves[m] = halves
                stats = tmpp.tile([P, 2, 6], f32, tag="bnstats")
                for t in range(NT):
                    nc.tensor.matmul(
                        halves[t],
                        wp_sb[:, KC - 1, m * P : (m + 1) * P],
                        a_u[:, KC - 1, t * 512 : (t + 1) * 512],
                        start=False, stop=True,
                        skip_group_check=True,
                    )
                    nc.vector.bn_stats(out=stats[:, t, :], in_=halves[t])
                nc.vector.bn_aggr(out=mvs2[:, m, :], in_=stats[:])
            for m in range(KC):
                # m2 goes through two qp half-banks so it never waits on an
                # earlier chunk's apply to free stp; m3 recycles the first
                # freed stp buffer.
                if m in preruns:
                    halves = pr_halves[m]
                elif m == 2:
                    halves = []
                    for _t in range(NT):
                        psh = qp.tile([P, 512], f32, tag="qp")
                        halves.append(psh[:])
                else:
                    psm = stp.tile([P, N], f32, tag="stp")
                    halves = [psm[:, t * 512 : (t + 1) * 512] for t in range(NT)]
                if m not in preruns:
                    stats = tmpp.tile([P, 2, 6], f32, tag="bnstats")
                    for t in range(NT):
                        for k in range(KC):
                            nc.tensor.matmul(
                                halves[t],
                                wp_sb[:, k, m * P : (m + 1) * P],
                                a_u[:, k, t * 512 : (t + 1) * 512],
                                start=(k == 0),
                                stop=(k == KC - 1),
                                skip_group_check=True,
                            )
                        # stats on the finished half while the other runs
                        nc.vector.bn_stats(out=stats[:, t, :], in_=halves[t])
                    nc.vector.bn_aggr(out=mvs2[:, m, :], in_=stats[:])
                gn_chunk(m, mvs2[:, m, :], g2s, g2b, ab2,
                         bias_fold=bp_sb[:, m : m + 1])
                for t in range(NT):
                    nc.scalar.activation(
                        out=projf[:, m, t * 512 : (t + 1) * 512], in_=halves[t],
                        func=AF.Identity,
                        scale=ab2[:, m, 0:1], bias=ab2[:, m, 1:2],
                    )
                nc.gpsimd.dma_start(
                    out_d[:, m, :], projf[:, m, :],
                    accum_op=OP.add,
                )

    nc.compile()
    return nc


def _host_prep(x, gn1_scale, gn1_bias, w_qkv, b_qkv, w_proj, b_proj, gn2_scale, gn2_bias):
    """Build per-core input maps (numpy only)."""
    f = np.float32
    bf = np.float16
    x = np.asarray(x, f)
    w_qkv = np.asarray(w_qkv, f)
    b_qkv = np.asarray(b_qkv, f)
    w_proj = np.asarray(w_proj, f)
    b_proj = np.asarray(b_proj, f)
    gn1_scale = np.asarray(gn1_scale, f)
    gn1_bias = np.asarray(gn1_bias, f)
    gn2_scale = np.asarray(gn2_scale, f)
    gn2_bias = np.asarray(gn2_bias, f)

    def chunk_vec(v):  # [C] -> [P, KC]
        return np.ascontiguousarray(v.reshape(KC, P).T)

    def chunk_mat(wt, dt=f):  # [C, M] -> [P, KC, M]
        return np.ascontiguousarray(wt.reshape(KC, P, -1).transpose(1, 0, 2).astype(dt))

    idx = np.arange(NHEAD)[:, None] * (3 * CH) + np.arange(CH)[None, :]
    q_idx, k_idx, v_idx = idx.ravel(), (idx + CH).ravel(), (idx + 2 * CH).ravel()

    s1 = float(CH) ** -0.25
    def mtile(w):  # [P, KC, C] -> [P, M, KC, P]
        return np.ascontiguousarray(
            w.reshape(P, KC, KC, P).transpose(0, 2, 1, 3)
        )
    wq = mtile(chunk_mat(w_qkv[q_idx].T * s1, bf))
    wk = mtile(chunk_mat(w_qkv[k_idx].T * s1, bf))
    wv = chunk_mat(w_qkv[v_idx].T, bf)
    wp = chunk_mat(w_proj.T, bf)
    wqk = np.ascontiguousarray(np.stack([wq, wk], axis=2))

    WI = 2 * KC * C
    WT = WI + P + P // 2
    wvp = np.zeros((P, WT), bf)
    wvp[:, 0 : KC * C] = wv.reshape(P, -1)
    wvp[:, KC * C : WI] = wp.reshape(P, -1)
    wvp[:, WI : WI + P] = np.eye(P, dtype=bf)
    # fp8e4m3 -I bit-packed into f16 lanes: -1.0 is 0xB8
    ni = np.zeros((P, P), np.uint8)
    ni[np.arange(P), np.arange(P)] = 0xB8
    wvp[:, WI + P : WT] = ni.view(np.uint16).view(bf)

    bq = chunk_vec(b_qkv[q_idx] * s1)
    bk = chunk_vec(b_qkv[k_idx] * s1)
    # v-bias folded into the proj bias: proj(a + bv) = proj(a) + w_proj @ bv
    bp_eff = b_proj + w_proj @ b_qkv[v_idx]
    bp = chunk_vec(bp_eff)

    cidx = np.arange(C)
    sel8 = np.zeros((P, KC, GC), f)
    sel8[cidx % P, cidx // P, (cidx % P) // 16] = 1.0 / 16.0
    selt8 = np.zeros((GC, KC, P), f)
    selt8[(cidx % P) // 16, cidx // P, cidx % P] = 1.0
    sel32 = np.zeros((P, KC, GROUPS), f)
    sel32[cidx % P, cidx // P, cidx // 16] = 1.0 / 16.0
    selt32 = np.zeros((GROUPS, KC, P), f)
    selt32[cidx // 16, cidx // P, cidx % P] = 1.0

    pars = np.zeros((P, 448), f)
    pars[:, 0:4] = chunk_vec(gn1_scale)
    pars[:, 4:8] = chunk_vec(gn1_bias)
    pars[:, 8:12] = bq
    pars[:, 12:16] = bk
    pars[:, 16:20] = bp
    pars[:, 20:24] = chunk_vec(gn2_scale)
    pars[:, 24:28] = chunk_vec(gn2_bias)
    pars[:, 28:60] = sel8.reshape(P, -1)
    pars[:, 60:188] = sel32.reshape(P, -1)
    pars[:, 444] = 1.0
    parsg = np.zeros((GROUPS, 1544), f)
    parsg[:, 0:512] = selt32.reshape(GROUPS, -1)
    parsg[:, 1024] = EPS
    parsg[0:GC, 1028:1540] = selt8.reshape(GC, -1)

    shared = {
        "wqk": wqk, "wvp": wvp, "pars": pars, "parsg": parsg,
    }
    in_maps = []
    for b in range(B):
        xb = np.ascontiguousarray(
            x[b].reshape(C, N).reshape(KC, P, N).transpose(1, 0, 2)
        )
        in_maps.append({"x": xb, "x8": xb.astype(bf), **shared})
    return in_maps


def _assemble(results):
    out = np.empty((B, C, H, W), np.float32)
    for b in range(B):
        ob = np.asarray(results[b]["out"])  # [P, KC, N]
        out[b] = ob.transpose(1, 0, 2).reshape(C, N).reshape(C, H, W)
    return out


def get_nc():
    if "nc" not in _CACHE:
        _CACHE["nc"] = _build_nc()
    return _CACHE["nc"]


def kernel(x, gn1_scale, gn1_bias, w_qkv, b_qkv, w_proj, b_proj, gn2_scale, gn2_bias):
    from concourse.bass_utils import run_bass_kernel_spmd

    nc = get_nc()
    in_maps = _host_prep(
        x, gn1_scale, gn1_bias, w_qkv, b_qkv, w_proj, b_proj, gn2_scale, gn2_bias
    )
    res = run_bass_kernel_spmd(nc, in_maps, core_ids=list(range(B)))
    return _assemble(res.results)


# revision 56
# speedup vs baseline: 1.0043x; 1.0043x over previous
"""Attention2d Trainium2 Bass kernel.

Reference computation (per batch element b of 8, one NeuronCore each):
    hn  = GroupNorm32(x) * gn1_scale + gn1_bias
    qkv = w_qkv @ hn + b_qkv          (1x1 conv == matmul over channels)
    per head h (8 heads, ch=64): q,k,v from qkv (torch reshape convention:
        head h uses rows h*192+{0..64,64..128,128..192})
    wgt = softmax((q*s)^T (k*s)), s = ch**-0.25
    a   = v @ wgt^T
    out = GroupNorm32(w_proj @ a + b_proj) ... * gn2_scale + gn2_bias
    y   = x + out

Device strategy (data-parallel over batch, 1 core per batch element):
  - ACT is the critical engine: the 8 heads' exp(S^T) stream is 64 tiles of
    [128, 1024] (~1.04us each). Everything else is scheduled so that stream
    never starves: PE work is held well under the stream duration.
  - S^T runs as fp8e4 DoubleRow matmuls at 0.5 cycles/row. To keep the
    quantization error inside the rel-err budget, q ships as TWO fp8
    k-subtiles (hi + residual lo, together fp16-accurate) against a
    DUPLICATED fp8 k: S = k8^T(q_hi + q_lo), so only k's single fp8
    rounding touches the logits. q_lo is formed by accumulating -I @ q_hi
    into the conv PSUM (one cheap fp8 matmul) and re-evacuating.
  - exps, v^T tiles, the A matmuls, convs and proj all stay fp16: their
    quantization hits the output directly (measured: fp8 exps alone cost
    2.6e-2 rel err), while the k-side fp8 is dampened through softmax.
  - the A matmul runs TRANSPOSED: out a^T[t-part, ch] per 128-wide t-chunk;
    the softmax denominator (ones-column of v^T) lands in column 64 as a
    per-partition scalar: one DVE reciprocal + one fused tensor_scalar
    normalizes while evacuating. The LAST head's A accumulation is split
    into s-halves so only half of it trails the final exp tile.
  - a^T -> a via XBAR dma_start_transpose (no engine time); the LAST pair
    uses PE identity-transposes so proj isn't tail-gated by the HWDGE queue
  - v-bias folded into the proj bias on the host; proj bias folded into the
    GN2 affine + channel stats
  - GN2: DVE bn_stats reads each proj PSUM tile directly (one pass), the
    group reduce + affine run on PE/ACT (idle post-stream), and a single
    ACT Identity applies y = ps*A + B straight out of PSUM
  - input DMAs are packed and ordered so GN1 stats chase the x chunks
    (ACT takes the chunk that lands first, DVE the rest), pair-0 weights
    ship in their own small DMA, and pair-0's q/k evacuations run on the
    still-idle ACT so the first exp fires as early as possible
  - scratch warm-up matmuls on a memset tile hold the PE in its fast
    p-state from t~0
"""

import numpy as np

NHEAD = 8
GROUPS = 32
EPS = 1e-5
B, C, H, W = 8, 512, 32, 32
N = H * W            # 1024 spatial positions
CH = C // NHEAD      # 64 channels per head
P = 128              # partitions
KC = C // P          # 4 channel chunks
NT = N // 512        # 2 column tiles of 512
SC = N // P          # 8 s-chunks
GC = GROUPS // KC    # 8 groups per chunk

_CACHE = {}


def _build_nc():
    import concourse.tile as tile
    from concourse import mybir, bacc
    from concourse.hw_specs import get_activation_tables

    f32 = mybir.dt.float32
    f16 = mybir.dt.float16
    f8 = mybir.dt.float8e4
    i16 = mybir.dt.int16
    AF = mybir.ActivationFunctionType
    OP = mybir.AluOpType
    PM = mybir.MatmulPerfMode

    nc = bacc.Bacc("TRN2", target_bir_lowering=False, num_devices=8)

    WI = 2 * KC * C          # wv/wp block columns in the packed weight tile
    WT = WI + P + P // 2     # + f16 identity + fp8 -identity (bit-packed)

    x_d = nc.dram_tensor("x", [P, KC, N], f32, kind="ExternalInput")
    x8_d = nc.dram_tensor("x8", [P, KC, N], f16, kind="ExternalInput")
    wqk_d = nc.dram_tensor("wqk", [P, KC, 2, KC, P], f16, kind="ExternalInput")
    wvp_d = nc.dram_tensor("wvp", [P, WT], f16, kind="ExternalInput")
    pars_d = nc.dram_tensor("pars", [P, 448], f32, kind="ExternalInput")
    parsg_d = nc.dram_tensor("parsg", [GROUPS, 1544], f32, kind="ExternalInput")
    out_d = nc.dram_tensor("out", [P, KC, N], f32, kind="ExternalOutput")

    with tile.TileContext(nc) as tc:
        with (
            tc.tile_pool(name="big", bufs=1) as big,
            tc.tile_pool(name="wpool", bufs=2) as wpool,
            tc.tile_pool(name="qpool", bufs=2) as qpool,
            tc.tile_pool(name="kpool", bufs=4) as kpool,
            tc.tile_pool(name="vtp", bufs=1) as vtp,
            tc.tile_pool(name="expp", bufs=3) as expp,
            tc.tile_pool(name="autp", bufs=2) as autp,
            tc.tile_pool(name="tmpp", bufs=2) as tmpp,
            tc.tile_pool(name="stp", bufs=2, space="PSUM") as stp,
            tc.tile_pool(name="apool", bufs=2, space="PSUM") as apool,
            tc.tile_pool(name="qp", bufs=2, space="PSUM") as qp,
        ):
            # ---------- persistent SBUF tiles ----------
            x_sb = big.tile([P, KC, N], f16, tag="x_sb")
            hn = big.tile([P, KC, N], f16, tag="hn")
            vt = vtp.tile([P, SC, NHEAD, CH + 1], f16, tag="vp")
            pars_sb = big.tile([P, 448], f32, tag="pars_sb")
            parsg_sb = big.tile([GROUPS, 1544], f32, tag="parsg_sb")
            ab1 = big.tile([P, KC, 2], f32, tag="ab1")
            ab2 = big.tile([P, KC, 2], f32, tag="ab2")
            projf = big.tile([P, KC, N], f32, tag="projf")
            a_u = big.tile([P, KC, N], f16, tag="a_u")
            scr16 = big.tile([P, 2, N], f16, tag="scr16")
            wmup = big.tile([P, 512], f16, tag="wmup")

            # packed-parameter views
            g1s = pars_sb[:, 0:4]
            g1b = pars_sb[:, 4:8]
            bq_sb = pars_sb[:, 8:12]
            bk_sb = pars_sb[:, 12:16]
            bp_sb = pars_sb[:, 16:20]
            g2s = pars_sb[:, 20:24]
            g2b = pars_sb[:, 24:28]
            sel8 = pars_sb[:, 28:60].rearrange("p (k g) -> p k g", k=KC)
            sel32 = pars_sb[:, 60:188].rearrange("p (k g) -> p k g", k=KC)
            onescol = pars_sb[:, 444:445]
            selt32 = parsg_sb[:, 0:512].rearrange("g (k c) -> g k c", k=KC)
            selt8 = parsg_sb[0:GC, 1028:1540].rearrange("g (k c) -> g k c", k=KC)
            eps32 = parsg_sb[:, 1024:1025]
            eps8 = parsg_sb[0:GC, 1024:1025]

            # PE warm-up from t~0: matmuls on a Pool-memset scratch tile hold
            # the PE through its p-state ramp so the first real matmuls run at
            # full speed. Results are discarded.
            nc.gpsimd.memset(wmup[:], 0.0)
            # enough back-to-back warm-up matmuls to bridge to the first GN1
            # group matmuls (~8us) -- a >~2us PE idle gap resets the ramp
            for _ in range(26):
                ps_w = qp.tile([P, 512], f32, tag="qp")
                nc.tensor.matmul(
                    ps_w[:, 0:448],
                    wmup[:, 0:128],
                    wmup[:, 0:448],
                    start=True, stop=True,
                )

            # ---------- input DMAs, ordered for the GN1 -> conv chain -------
            # chunk 2 lands first (its stats run on ACT), then chunk 3 so the
            # DVE's last bn_stats isn't the straggler; pair-0 weights ship in
            # their own small contiguous DMA.
            nc.sync.dma_start(x_sb[:, 2, :], x8_d[:, 2, :])
            nc.sync.dma_start(x_sb[:, 3, :], x8_d[:, 3, :])
            nc.sync.dma_start(x_sb[:, 0, :], x8_d[:, 0, :])
            nc.sync.dma_start(x_sb[:, 1, :], x8_d[:, 1, :])
            nc.sync.dma_start(pars_sb[:], pars_d[:])
            nc.sync.dma_start(parsg_sb[:], parsg_d[:])
            wqk_sb = wpool.tile([P, KC, 2, KC, P], f16, tag="wqk")
            nc.sync.dma_start(wqk_sb[:, 0], wqk_d[:, 0])
            nc.sync.dma_start(wqk_sb[:, 1:KC], wqk_d[:, 1:KC])
            wvp_sb = wpool.tile([P, WT], f16, tag="wvp")
            nc.sync.dma_start(wvp_sb[:], wvp_d[:])
            nc.sync.dma_start(out_d[:], x_d[:])

            wq_sb = wqk_sb[:, :, 0]
            wk_sb = wqk_sb[:, :, 1]
            wv_sb = wvp_sb[:, 0 : KC * C].rearrange("p (k c) -> p k c", k=KC)
            wp_sb = wvp_sb[:, KC * C : WI].rearrange("p (k c) -> p k c", k=KC)
            ident_sb = wvp_sb[:, WI : WI + P]
            negid8 = wvp_sb[:, WI + P : WT].bitcast(f8)

            # Preload the combined ln+exp ACT table set once (Ln/Exp are used
            # for the GroupNorm rstd), so the bacc table-load pass doesn't
            # thrash between sets.
            _set_names = list(get_activation_tables(nc.m.arch).keys())
            _tl = mybir.InstLoadActFuncSet(
                name=nc.get_next_instruction_name(),
                ins=[],
                outs=[],
                act_func_set_id=_set_names.index("natural_log_exp_and_others"),
            )
            _tl.engine = mybir.EngineType.Activation
            nc.scalar.add_instruction(_tl)

            nc.gpsimd.tensor_copy(
                out=vt[:, :, :, CH : CH + 1],
                in_=onescol[:, :, None, None].to_broadcast((P, SC, NHEAD, 1)),
            )

            # ---------- per-chunk GroupNorm chain (used by GN2) ----------
            def gn_chunk(k, mvs_ap, gs, gb, ab, bias_fold):
                """ab[:, k] = per-channel (A, B) for y = src*A + B, given
                per-channel (mean, var) in mvs_ap ([P, 2], chunk k); the
                [P, 1] AP bias_fold adjusts the stats and B as if it had been
                added to the source. DVE ops read the group-reduce PSUM
                directly to keep the dependency chain short."""
                stat2 = tmpp.tile([P, 2], f32, tag="stat2")
                nc.vector.tensor_tensor(
                    stat2[:, 0:1], mvs_ap[:, 0:1], bias_fold, OP.add
                )
                musq = tmpp.tile([P, 1], f32, tag="musq")
                nc.vector.tensor_tensor(musq[:], stat2[:, 0:1], stat2[:, 0:1], OP.mult)
                nc.vector.tensor_tensor(stat2[:, 1:2], mvs_ap[:, 1:2], musq[:], OP.add)
                ps_g = apool.tile([P, 260], f32, tag="apool")
                nc.tensor.matmul(
                    ps_g[0:GC, 0:2], sel8[:, k, :], stat2[:, :],
                    start=True, stop=True,
                )
                gstat = tmpp.tile([GC, 2], f32, tag="gstat")
                nc.vector.tensor_copy(out=gstat[:, 0:1], in_=ps_g[0:GC, 0:1])
                gvar = tmpp.tile([GC, 1], f32, tag="gvar")
                gmusq = tmpp.tile([GC, 1], f32, tag="gmusq")
                # only one PSUM operand allowed per instruction: square the
                # SBUF copy of the group mean
                nc.vector.tensor_tensor(gmusq[:], gstat[:, 0:1], gstat[:, 0:1], OP.mult)
                nc.vector.tensor_tensor(gvar[:], ps_g[0:GC, 1:2], gmusq[:], OP.subtract)
                nc.scalar.activation(out=gvar[:], in_=gvar[:], func=AF.Ln, bias=eps8)
                nc.scalar.activation(out=gstat[:, 1:2], in_=gvar[:], func=AF.Exp, scale=-0.5)
                ps_c = apool.tile([P, 260], f32, tag="apool")
                nc.tensor.matmul(
                    ps_c[:, 0:2], selt8[:, k, :], gstat[:],
                    start=True, stop=True,
                )
                nc.vector.tensor_tensor(
                    ab[:, k, 0:1], gs[:, k : k + 1], ps_c[:, 1:2], OP.mult
                )
                # B = gb + A*(bias_fold - mean_c)
                ma = tmpp.tile([P, 2], f32, tag="ma")
                nc.vector.tensor_tensor(ma[:, 0:1], bias_fold, ps_c[:, 0:1], OP.subtract)
                nc.vector.tensor_tensor(ma[:, 1:2], ab[:, k, 0:1], ma[:, 0:1], OP.mult)
                nc.vector.tensor_tensor(
                    ab[:, k, 1:2], gb[:, k : k + 1], ma[:, 1:2], OP.add
                )

            # ---------- GN1 -> hn (stats chase the x chunk DMAs: ACT
            # accumulators for chunk 2 (lands first), DVE bn_stats for chunks
            # 3,0,1 in arrival order; one batched group reduce + affine) -----
            BN_CHUNKS = (3, 0, 1)
            mvs1 = big.tile([P, 3, 2], f32, tag="mvs1")
            stat2 = big.tile([P, KC, 2], f32, tag="stat2b")
            nc.scalar.activation(
                out=scr16[:, 0, :], in_=x_sb[:, 2, :], func=AF.Copy,
                scale=1.0 / N, accum_out=stat2[:, 2, 0:1],
            )
            nc.scalar.activation(
                out=scr16[:, 1, :], in_=x_sb[:, 2, :], func=AF.Square,
                scale=1.0 / 32.0, accum_out=stat2[:, 2, 1:2],
            )
            for i, k in enumerate(BN_CHUNKS):
                stats = tmpp.tile([P, 2, 6], f32, tag="bnstats")
                resh = x_sb[:, k, :].rearrange("p (s f) -> p s f", f=512)
                for si in range(2):
                    nc.vector.bn_stats(out=stats[:, si, :], in_=resh[:, si, :])
                nc.vector.bn_aggr(out=mvs1[:, i, :], in_=stats[:])
            musq = tmpp.tile([P, 3], f32, tag="musqb")
            nc.vector.tensor_tensor(musq[:], mvs1[:, :, 0], mvs1[:, :, 0], OP.mult)
            for i, k in enumerate(BN_CHUNKS):
                nc.vector.tensor_tensor(
                    stat2[:, k, 1:2], mvs1[:, i, 1:2], musq[:, i : i + 1], OP.add
                )
                nc.vector.tensor_copy(out=stat2[:, k, 0:1], in_=mvs1[:, i, 0:1])
            ps_g1 = qp.tile([P, 512], f32, tag="qp")
            for k in range(KC):
                nc.tensor.matmul(
                    ps_g1[0:GROUPS, 0:2], sel32[:, k, :], stat2[:, k, :],
                    start=(k == 0), stop=(k == KC - 1),
                )
            gst1 = big.tile([GROUPS, 2], f32, tag="gst1")
            gms1 = tmpp.tile([GROUPS, 2], f32, tag="gms1")
            nc.vector.tensor_copy(out=gms1[:], in_=ps_g1[0:GROUPS, 0:2])
            nc.vector.tensor_copy(out=gst1[:, 0:1], in_=gms1[:, 0:1])
            gv1 = tmpp.tile([GROUPS, 1], f32, tag="gv1")
            gmu1 = tmpp.tile([GROUPS, 1], f32, tag="gmu1")
            nc.vector.tensor_tensor(gmu1[:], gms1[:, 0:1], gms1[:, 0:1], OP.mult)
            nc.vector.tensor_tensor(gv1[:], gms1[:, 1:2], gmu1[:], OP.subtract)
            nc.scalar.activation(out=gv1[:], in_=gv1[:], func=AF.Ln, bias=eps32)
            nc.scalar.activation(out=gst1[:, 1:2], in_=gv1[:], func=AF.Exp, scale=-0.5)
            ps_c1 = qp.tile([P, 512], f32, tag="qp")
            for k in range(KC):
                nc.tensor.matmul(
                    ps_c1[:, 2 * k : 2 * k + 2], selt32[:, k, :], gst1[:],
                    start=True, stop=True,
                )
            cst1 = tmpp.tile([P, KC, 2], f32, tag="cst1")
            nc.vector.tensor_copy(out=cst1[:], in_=ps_c1[:, 0 : 2 * KC])
            nc.vector.tensor_tensor(ab1[:, :, 0], g1s[:, :], cst1[:, :, 1], OP.mult)
            ma1 = tmpp.tile([P, KC], f32, tag="ma1")
            nc.vector.tensor_tensor(ma1[:], cst1[:, :, 0], ab1[:, :, 0], OP.mult)
            nc.vector.tensor_tensor(ab1[:, :, 1], g1b[:, :], ma1[:], OP.subtract)
            for k in range(KC):
                nc.vector.tensor_scalar(
                    hn[:, k, :], x_sb[:, k, :],
                    ab1[:, k, 0:1], ab1[:, k, 1:2], OP.mult, OP.add,
                )

            # ---------- phase helpers ----------
            def evac(eng, dst, src, bias_ap):
                if eng is None:
                    # ACT bias-add copy (idle pre-stream)
                    nc.scalar.activation(
                        out=dst, in_=src, func=AF.Identity, bias=bias_ap
                    )
                else:
                    eng.tensor_scalar(dst, src, bias_ap, None, OP.add)

            def conv_q(p, qt, eng):
                # q as hi+lo fp8 pair: evac hi, subtract rne8(hi) from the
                # PSUM via an accumulated -I @ hi matmul, evac the residual
                # on the (otherwise idle) Pool so the DVE keeps up with the
                # Schraudolph share of the exp stream.
                bias_ap = bq_sb[:, p : p + 1]
                for t in range(NT):
                    ts_ = slice(t * 512, (t + 1) * 512)
                    ps = qp.tile([P, 512], f32, tag="qp")
                    for k in range(KC):
                        nc.tensor.matmul(
                            ps[:, :],
                            wq_sb[:, p, k, :],
                            hn[:, k, ts_],
                            start=(k == 0), stop=(k == KC - 1),
                        )
                    evac(eng, qt[:, 0, ts_], ps[:], bias_ap)
                    nc.tensor.matmul(
                        ps[:, :], negid8[:], qt[:, 0, ts_],
                        start=False, stop=True, skip_group_check=True,
                    )
                    # pair 0's lo goes to the DVE (free once hn is done) so
                    # the ACT-side hi evacs and the first exps aren't queued
                    # behind it
                    evac(nc.vector if eng is None else eng,
                         qt[:, 1, ts_], ps[:], bias_ap)

            def conv_k(p, kt, eng, trange=range(NT)):
                bias_ap = bk_sb[:, p : p + 1]
                for t in trange:
                    ts_ = slice(t * 512, (t + 1) * 512)
                    ps = qp.tile([P, 512], f32, tag="qp")
                    for k in range(KC):
                        nc.tensor.matmul(
                            ps[:, :],
                            wk_sb[:, p, k, :],
                            hn[:, k, ts_],
                            start=(k == 0), stop=(k == KC - 1),
                        )
                    evac(eng, kt[:, 0, ts_], ps[:], bias_ap)
                    # duplicate into subtile 1 for the DoubleRow layout
                    nc.gpsimd.tensor_copy(out=kt[:, 1, ts_], in_=kt[:, 0, ts_])

            def v_tiles(half):
                # v^T tiles [s-part, head-major channel]; bv is folded into
                # the proj bias on the host, so no bias row here. Built in
                # halves slotted into heads 0 and 1 so the DVE evacuation
                # copies don't pile up in one stream window.
                for nt in range(4 * half, 4 * half + 4):
                    ps = qp.tile([P, 512], f32, tag="qp")
                    for k in range(KC):
                        nc.tensor.matmul(
                            ps[:, :],
                            hn[:, k, nt * P : (nt + 1) * P],
                            wv_sb[:, k, :],
                            start=(k == 0), stop=(k == KC - 1),
                        )
                    nc.vector.tensor_copy(
                        out=vt[:, nt, :, 0:CH],
                        in_=ps[:, :].rearrange("p (h c) -> p h c", h=NHEAD),
                    )

            # s-chunks routed to the DVE via the Schraudolph 2^x bit trick:
            # i16 = round(S*1024*log2(e) + (15*1024 - 62.2)) reinterpreted as
            # fp16 approximates exp(S) to ~+-4% -- the softmax denominator
            # uses the same approximated values, so the common mode cancels.
            SCHRA = (2, 5)
            SC1 = 1024 * 1.4426950408889634
            SC2 = 15360.0 - 62.2

            def head_st(h, qt, kt, mid=None):
                # S^T as fp8 DoubleRow: subtiles = (q_hi, q_lo) against a
                # duplicated k8, so each [128, 512] output costs 256 PE
                # cycles at near-fp16 accuracy.
                p, e = h // 2, h % 2
                rows = slice(64 * e, 64 * e + 64)
                exps = expp.tile([P, SC, N], f16, tag="exps")
                for sc in range(SC):
                    if sc == 2 and mid is not None:
                        mid()
                    if sc in SCHRA:
                        # Schraudolph tiles go through qp halves so the
                        # ACT-fed stp pipeline never waits on the DVE
                        for t in range(NT):
                            pq = qp.tile([P, 512], f32, tag="qp")
                            nc.tensor.matmul(
                                pq[:, :],
                                kt[rows, :, sc * P : (sc + 1) * P],
                                qt[rows, :, t * 512 : (t + 1) * 512],
                                start=True, stop=True,
                                perf_mode=PM.DoubleRow,
                            )
                            nc.vector.tensor_scalar(
                                exps[:, sc, t * 512 : (t + 1) * 512].bitcast(i16),
                                pq[:], SC1, SC2, OP.mult, OP.add,
                            )
                        continue
                    ps_st = stp.tile([P, N], f32, tag="stp")
                    for t in range(NT):
                        nc.tensor.matmul(
                            ps_st[:, t * 512 : (t + 1) * 512],
                            kt[rows, :, sc * P : (sc + 1) * P],
                            qt[rows, :, t * 512 : (t + 1) * 512],
                            start=True, stop=True,
                            perf_mode=PM.DoubleRow,
                        )
                    nc.scalar.activation(
                        out=exps[:, sc, :], in_=ps_st[:], func=AF.Exp
                    )
                return exps

            def head_a(h, exps, auT):
                # Transposed A: out a^T[t-part, ch] per 128-wide t-chunk, the
                # softmax denominator lands in column 64 as a per-partition
                # scalar -> one reciprocal + a fused normalize-evacuate.
                # (NOTE: the s accumulation must NOT interleave j groups --
                # PSUM allows one pending accumulation group per bank.)
                e = h % 2
                for u in range(2):
                    ps_aT = apool.tile([P, 260], f32, tag="apool")
                    pv = ps_aT[:].rearrange("p (j c) -> p j c", c=65)
                    for j in range(4):
                        tch = 4 * u + j
                        for sc in range(SC):
                            nc.tensor.matmul(
                                pv[:, j, :],
                                exps[:, sc, tch * P : (tch + 1) * P],
                                vt[:, sc, h, :],
                                start=(sc == 0), stop=(sc == SC - 1),
                            )
                    rcol = tmpp.tile([P, 4], f32, tag="rcol")
                    nc.vector.reciprocal(rcol[:], pv[:, :, 64])
                    for j in range(4):
                        tch = 4 * u + j
                        nc.vector.tensor_scalar(
                            auT[:, tch, 64 * e : 64 * e + 64], pv[:, j, 0:64],
                            rcol[:, j : j + 1], None, OP.mult,
                        )

            # ---------- qkv, then attention ----------
            def conv_pair(p, eng):
                qt = qpool.tile([P, 2, N], f8, tag="qt")
                kt = kpool.tile([P, 2, N], f8, tag="kt")
                conv_k(p, kt, eng, trange=(0,))
                conv_q(p, qt, eng)
                conv_k(p, kt, eng, trange=(1,))
                return qt, kt

            # pair 0 evacuates on the still-idle ACT so the stream starts
            # as early as possible; later pairs use the DVE.
            qt, kt = conv_pair(0, None)
            e = {}
            e[0] = head_st(0, qt, kt, mid=lambda: v_tiles(0))
            # v^T tiles build on the PE while head 0/1's exps stream, slotted
            # into the middles of both heads' S^T so exp starts on time
            e[1] = head_st(1, qt, kt, mid=lambda: v_tiles(1))
            auTs = {}

            def transposes(p):
                # chunks 0/1 ride the XBAR mid-stream; chunks 2/3 take PE
                # identity-transposes -- the XBAR's HWDGE serialization
                # (~5us for 8 tiles + ~1us completion sem) would gate the
                # proj pre-runs right at the stream tail
                if p < KC - 2:
                    # XBAR transpose a^T -> a_u chunk p (SBUF->SBUF, no
                    # PE/DVE time)
                    for tch in range(SC):
                        nc.sync.dma_start_transpose(
                            a_u[:, p, tch * P : (tch + 1) * P], auTs[p][:, tch, :]
                        )
                else:
                    # tail chunk: PE transposes (identity matmul) -> shortest
                    # path into proj's k=3 contraction; the PSUM->SBUF copies
                    # run on the post-stream-idle ACT so the DVE (busy with
                    # the last head's normalize) isn't the serializer
                    for tch in range(SC):
                        ps_t = apool.tile([P, 260], f32, tag="apool")
                        pt = ps_t[:].bitcast(f16)
                        nc.tensor.matmul(
                            pt[:, 0:P], auTs[p][:, tch, :], ident_sb[:],
                            is_transpose=True,
                        )
                        if p == KC - 1:
                            # post-stream: ACT is idle, keep the DVE free for
                            # the bn/gn chains
                            nc.scalar.activation(
                                out=a_u[:, p, tch * P : (tch + 1) * P],
                                in_=pt[:, 0:P], func=AF.Copy,
                            )
                        else:
                            nc.vector.tensor_copy(
                                out=a_u[:, p, tch * P : (tch + 1) * P],
                                in_=pt[:, 0:P],
                            )

            # software pipeline: S^T/exp of pair p streams while the A
            # matmuls of pair p-1 drain, so pair boundaries stay dense
            for p in range(1, KC):
                qt, kt = conv_pair(p, nc.vector)
                h0 = 2 * (p - 1)
                e[2 * p] = head_st(2 * p, qt, kt)
                auT = autp.tile([P, SC, P], f16, tag="auT")
                auTs[p - 1] = auT
                head_a(h0, e[h0], auTs[p - 1])
                e[2 * p + 1] = head_st(2 * p + 1, qt, kt)
                head_a(h0 + 1, e[h0 + 1], auTs[p - 1])
                transposes(p - 1)
            auT = autp.tile([P, SC, P], f16, tag="auT")
            auTs[KC - 1] = auT
            head_a(6, e[6], auTs[KC - 1])

            head_a(7, e[7], auTs[KC - 1])

            # proj m0's k<3 accumulation also pre-runs under the stream tail
            preruns = {}
            ps_pm0 = stp.tile([P, N], f32, tag="stp")
            preruns[0] = ps_pm0
            for t in range(NT):
                for k in range(KC - 1):
                    nc.tensor.matmul(
                        preruns[0][:, t * 512 : (t + 1) * 512],
                        wp_sb[:, k, 0:P],
                        a_u[:, k, t * 512 : (t + 1) * 512],
                        start=(k == 0), stop=False,
                        skip_group_check=True,
                    )

            transposes(KC - 1)

            # ---------- proj + GN2 + residual, pipelined per chunk ----------
            # DVE bn_stats reads the proj PSUM directly (one pass for mean
            # and var); the per-chunk group reduce + affine then run on the
            # post-stream-idle PE/ACT, and one ACT Identity applies
            # y = ps*A + B straight out of PSUM. The (host-folded) proj bias
            # enters the stats and the B term via gn_chunk's bias_fold.
            mvs2 = big.tile([P, KC, 2], f32, tag="mvs2")
            # m0 and m1 finish their pre-run accumulators and take their
            # stats back-to-back BEFORE either gn chain runs, so the second
            # bn pass isn't stuck behind the first gn's PE/ACT hops in the
            # in-order DVE queue
            pr_halves = {}
            for m in preruns:
                halves = [
                    preruns[m][:, t * 512 : (t + 1) * 512] for t in range(NT)
                ]
                pr_halves[m] = halves
                stats = tmpp.tile([P, 2, 6], f32, tag="bnstats")
                for t in range(NT):
                    nc.tensor.matmul(
                        halves[t],
                        wp_sb[:, KC - 1, m * P : (m + 1) * P],
                        a_u[:, KC - 1, t * 512 : (t + 1) * 512],
                        start=False, stop=True,
                        skip_group_check=True,
                    )
                    nc.vector.bn_stats(out=stats[:, t, :], in_=halves[t])
                nc.vector.bn_aggr(out=mvs2[:, m, :], in_=stats[:])
            for m in range(KC):
                # m2 goes through two qp half-banks so it never waits on an
                # earlier chunk's apply to free stp; m3 recycles the first
                # freed stp buffer.
                if m in preruns:
                    halves = pr_halves[m]
                elif m == 2:
                    halves = []
                    for _t in range(NT):
                        psh = qp.tile([P, 512], f32, tag="qp")
                        halves.append(psh[:])
                else:
                    psm = stp.tile([P, N], f32, tag="stp")
                    halves = [psm[:, t * 512 : (t + 1) * 512] for t in range(NT)]
                if m not in preruns:
                    stats = tmpp.tile([P, 2, 6], f32, tag="bnstats")
                    for t in range(NT):
                        for k in range(KC):
                            nc.tensor.matmul(
                                halves[t],
                                wp_sb[:, k, m * P : (m + 1) * P],
                                a_u[:, k, t * 512 : (t + 1) * 512],
                                start=(k == 0),
                                stop=(k == KC - 1),
                                skip_group_check=True,
                            )
                        # stats on the finished half while the other runs
                        nc.vector.bn_stats(out=stats[:, t, :], in_=halves[t])
                    nc.vector.bn_aggr(out=mvs2[:, m, :], in_=stats[:])
                gn_chunk(m, mvs2[:, m, :], g2s, g2b, ab2,
                         bias_fold=bp_sb[:, m : m + 1])
                for t in range(NT):
                    nc.scalar.activation(
                        out=projf[:, m, t * 512 : (t + 1) * 512], in_=halves[t],
                        func=AF.Identity,
                        scale=ab2[:, m, 0:1], bias=ab2[:, m, 1:2],
                    )
                for t in range(NT):
                    nc.gpsimd.dma_start(
                        out_d[:, m, t * 512 : (t + 1) * 512],
                        projf[:, m, t * 512 : (t + 1) * 512],
                        accum_op=OP.add,
                    )

    nc.compile()
    return nc


def _host_prep(x, gn1_scale, gn1_bias, w_qkv, b_qkv, w_proj, b_proj, gn2_scale, gn2_bias):
    """Build per-core input maps (numpy only)."""
    f = np.float32
    bf = np.float16
    x = np.asarray(x, f)
    w_qkv = np.asarray(w_qkv, f)
    b_qkv = np.asarray(b_qkv, f)
    w_proj = np.asarray(w_proj, f)
    b_proj = np.asarray(b_proj, f)
    gn1_scale = np.asarray(gn1_scale, f)
    gn1_bias = np.asarray(gn1_bias, f)
    gn2_scale = np.asarray(gn2_scale, f)
    gn2_bias = np.asarray(gn2_bias, f)

    def chunk_vec(v):  # [C] -> [P, KC]
        return np.ascontiguousarray(v.reshape(KC, P).T)

    def chunk_mat(wt, dt=f):  # [C, M] -> [P, KC, M]
        return np.ascontiguousarray(wt.reshape(KC, P, -1).transpose(1, 0, 2).astype(dt))

    idx = np.arange(NHEAD)[:, None] * (3 * CH) + np.arange(CH)[None, :]
    q_idx, k_idx, v_idx = idx.ravel(), (idx + CH).ravel(), (idx + 2 * CH).ravel()

    s1 = float(CH) ** -0.25
    def mtile(w):  # [P, KC, C] -> [P, M, KC, P]
        return np.ascontiguousarray(
            w.reshape(P, KC, KC, P).transpose(0, 2, 1, 3)
        )
    wq = mtile(chunk_mat(w_qkv[q_idx].T * s1, bf))
    wk = mtile(chunk_mat(w_qkv[k_idx].T * s1, bf))
    wv = chunk_mat(w_qkv[v_idx].T, bf)
    wp = chunk_mat(w_proj.T, bf)
    wqk = np.ascontiguousarray(np.stack([wq, wk], axis=2))

    WI = 2 * KC * C
    WT = WI + P + P // 2
    wvp = np.zeros((P, WT), bf)
    wvp[:, 0 : KC * C] = wv.reshape(P, -1)
    wvp[:, KC * C : WI] = wp.reshape(P, -1)
    wvp[:, WI : WI + P] = np.eye(P, dtype=bf)
    # fp8e4m3 -I bit-packed into f16 lanes: -1.0 is 0xB8
    ni = np.zeros((P, P), np.uint8)
    ni[np.arange(P), np.arange(P)] = 0xB8
    wvp[:, WI + P : WT] = ni.view(np.uint16).view(bf)

    bq = chunk_vec(b_qkv[q_idx] * s1)
    bk = chunk_vec(b_qkv[k_idx] * s1)
    # v-bias folded into the proj bias: proj(a + bv) = proj(a) + w_proj @ bv
    bp_eff = b_proj + w_proj @ b_qkv[v_idx]
    bp = chunk_vec(bp_eff)

    cidx = np.arange(C)
    sel8 = np.zeros((P, KC, GC), f)
    sel8[cidx % P, cidx // P, (cidx % P) // 16] = 1.0 / 16.0
    selt8 = np.zeros((GC, KC, P), f)
    selt8[(cidx % P) // 16, cidx // P, cidx % P] = 1.0
    sel32 = np.zeros((P, KC, GROUPS), f)
    sel32[cidx % P, cidx // P, cidx // 16] = 1.0 / 16.0
    selt32 = np.zeros((GROUPS, KC, P), f)
    selt32[cidx // 16, cidx // P, cidx % P] = 1.0

    pars = np.zeros((P, 448), f)
    pars[:, 0:4] = chunk_vec(gn1_scale)
    pars[:, 4:8] = chunk_vec(gn1_bias)
    pars[:, 8:12] = bq
    pars[:, 12:16] = bk
    pars[:, 16:20] = bp
    pars[:, 20:24] = chunk_vec(gn2_scale)
    pars[:, 24:28] = chunk_vec(gn2_bias)
    pars[:, 28:60] = sel8.reshape(P, -1)
    pars[:, 60:188] = sel32.reshape(P, -1)
    pars[:, 444] = 1.0
    parsg = np.zeros((GROUPS, 1544), f)
    parsg[:, 0:512] = selt32.reshape(GROUPS, -1)
    parsg[:, 1024] = EPS
    parsg[0:GC, 1028:1540] = selt8.reshape(GC, -1)

    shared = {
        "wqk": wqk, "wvp": wvp, "pars": pars, "parsg": parsg,
    }
    in_maps = []
    for b in range(B):
        xb = np.ascontiguousarray(
            x[b].reshape(C, N).reshape(KC, P, N).transpose(1, 0, 2)
        )
        in_maps.append({"x": xb, "x8": xb.astype(bf), **shared})
    return in_maps


def _assemble(results):
    out = np.empty((B, C, H, W), np.float32)
    for b in range(B):
        ob = np.asarray(results[b]["out"])  # [P, KC, N]
        out[b] = ob.transpose(1, 0, 2).reshape(C, N).reshape(C, H, W)
    return out


def get_nc():
    if "nc" not in _CACHE:
        _CACHE["nc"] = _build_nc()
    return _CACHE["nc"]


def kernel(x, gn1_scale, gn1_bias, w_qkv, b_qkv, w_proj, b_proj, gn2_scale, gn2_bias):
    from concourse.bass_utils import run_bass_kernel_spmd

    nc = get_nc()
    in_maps = _host_prep(
        x, gn1_scale, gn1_bias, w_qkv, b_qkv, w_proj, b_proj, gn2_scale, gn2_bias
    )
    res = run_bass_kernel_spmd(nc, in_maps, core_ids=list(range(B)))
    return _assemble(res.results)


# revision 61
# speedup vs baseline: 1.0100x; 1.0057x over previous
"""Attention2d Trainium2 Bass kernel.

Reference computation (per batch element b of 8, one NeuronCore each):
    hn  = GroupNorm32(x) * gn1_scale + gn1_bias
    qkv = w_qkv @ hn + b_qkv          (1x1 conv == matmul over channels)
    per head h (8 heads, ch=64): q,k,v from qkv (torch reshape convention:
        head h uses rows h*192+{0..64,64..128,128..192})
    wgt = softmax((q*s)^T (k*s)), s = ch**-0.25
    a   = v @ wgt^T
    out = GroupNorm32(w_proj @ a + b_proj) ... * gn2_scale + gn2_bias
    y   = x + out

Device strategy (data-parallel over batch, 1 core per batch element):
  - ACT is the critical engine: the 8 heads' exp(S^T) stream is 64 tiles of
    [128, 1024] (~1.04us each). Everything else is scheduled so that stream
    never starves: PE work is held well under the stream duration.
  - S^T runs as fp8e4 DoubleRow matmuls at 0.5 cycles/row. To keep the
    quantization error inside the rel-err budget, q ships as TWO fp8
    k-subtiles (hi + residual lo, together fp16-accurate) against a
    DUPLICATED fp8 k: S = k8^T(q_hi + q_lo), so only k's single fp8
    rounding touches the logits. q_lo is formed by accumulating -I @ q_hi
    into the conv PSUM (one cheap fp8 matmul) and re-evacuating.
  - exps, v^T tiles, the A matmuls, convs and proj all stay fp16: their
    quantization hits the output directly (measured: fp8 exps alone cost
    2.6e-2 rel err), while the k-side fp8 is dampened through softmax.
  - the A matmul runs TRANSPOSED: out a^T[t-part, ch] per 128-wide t-chunk;
    the softmax denominator (ones-column of v^T) lands in column 64 as a
    per-partition scalar: one DVE reciprocal + one fused tensor_scalar
    normalizes while evacuating. The LAST head's A accumulation is split
    into s-halves so only half of it trails the final exp tile.
  - a^T -> a via XBAR dma_start_transpose (no engine time); the LAST pair
    uses PE identity-transposes so proj isn't tail-gated by the HWDGE queue
  - v-bias folded into the proj bias on the host; proj bias folded into the
    GN2 affine + channel stats
  - GN2: DVE bn_stats reads each proj PSUM tile directly (one pass), the
    group reduce + affine run on PE/ACT (idle post-stream), and a single
    ACT Identity applies y = ps*A + B straight out of PSUM
  - input DMAs are packed and ordered so GN1 stats chase the x chunks
    (ACT takes the chunk that lands first, DVE the rest), pair-0 weights
    ship in their own small DMA, and pair-0's q/k evacuations run on the
    still-idle ACT so the first exp fires as early as possible
  - scratch warm-up matmuls on a memset tile hold the PE in its fast
    p-state from t~0
"""

import numpy as np

NHEAD = 8
GROUPS = 32
EPS = 1e-5
B, C, H, W = 8, 512, 32, 32
N = H * W            # 1024 spatial positions
CH = C // NHEAD      # 64 channels per head
P = 128              # partitions
KC = C // P          # 4 channel chunks
NT = N // 512        # 2 column tiles of 512
SC = N // P          # 8 s-chunks
GC = GROUPS // KC    # 8 groups per chunk

_CACHE = {}


def _build_nc():
    import concourse.tile as tile
    from concourse import mybir, bacc
    from concourse.hw_specs import get_activation_tables

    f32 = mybir.dt.float32
    f16 = mybir.dt.float16
    f8 = mybir.dt.float8e4
    i16 = mybir.dt.int16
    AF = mybir.ActivationFunctionType
    OP = mybir.AluOpType
    PM = mybir.MatmulPerfMode

    nc = bacc.Bacc("TRN2", target_bir_lowering=False, num_devices=8)

    WI = 2 * KC * C          # wv/wp block columns in the packed weight tile
    WT = WI + P + P // 2     # + f16 identity + fp8 -identity (bit-packed)

    x_d = nc.dram_tensor("x", [P, KC, N], f32, kind="ExternalInput")
    x8_d = nc.dram_tensor("x8", [P, KC, N], f16, kind="ExternalInput")
    wqk_d = nc.dram_tensor("wqk", [P, KC, 2, KC, P], f16, kind="ExternalInput")
    wvp_d = nc.dram_tensor("wvp", [P, WT], f16, kind="ExternalInput")
    pars_d = nc.dram_tensor("pars", [P, 448], f32, kind="ExternalInput")
    parsg_d = nc.dram_tensor("parsg", [GROUPS, 1544], f32, kind="ExternalInput")
    out_d = nc.dram_tensor("out", [P, KC, N], f32, kind="ExternalOutput")

    with tile.TileContext(nc) as tc:
        with (
            tc.tile_pool(name="big", bufs=1) as big,
            tc.tile_pool(name="wpool", bufs=2) as wpool,
            tc.tile_pool(name="qpool", bufs=2) as qpool,
            tc.tile_pool(name="kpool", bufs=4) as kpool,
            tc.tile_pool(name="vtp", bufs=1) as vtp,
            tc.tile_pool(name="expp", bufs=3) as expp,
            tc.tile_pool(name="autp", bufs=2) as autp,
            tc.tile_pool(name="tmpp", bufs=2) as tmpp,
            tc.tile_pool(name="stp", bufs=2, space="PSUM") as stp,
            tc.tile_pool(name="apool", bufs=2, space="PSUM") as apool,
            tc.tile_pool(name="qp", bufs=2, space="PSUM") as qp,
        ):
            # ---------- persistent SBUF tiles ----------
            x_sb = big.tile([P, KC, N], f16, tag="x_sb")
            hn = big.tile([P, KC, N], f16, tag="hn")
            vt = vtp.tile([P, SC, NHEAD, CH + 1], f16, tag="vp")
            pars_sb = big.tile([P, 448], f32, tag="pars_sb")
            parsg_sb = big.tile([GROUPS, 1544], f32, tag="parsg_sb")
            ab1 = big.tile([P, KC, 2], f32, tag="ab1")
            ab2 = big.tile([P, KC, 2], f32, tag="ab2")
            projf = big.tile([P, KC, N], f32, tag="projf")
            a_u = big.tile([P, KC, N], f16, tag="a_u")
            scr16 = big.tile([P, 2, N], f16, tag="scr16")
            wmup = big.tile([P, 512], f16, tag="wmup")

            # packed-parameter views
            g1s = pars_sb[:, 0:4]
            g1b = pars_sb[:, 4:8]
            bq_sb = pars_sb[:, 8:12]
            bk_sb = pars_sb[:, 12:16]
            bp_sb = pars_sb[:, 16:20]
            g2s = pars_sb[:, 20:24]
            g2b = pars_sb[:, 24:28]
            sel8 = pars_sb[:, 28:60].rearrange("p (k g) -> p k g", k=KC)
            sel32 = pars_sb[:, 60:188].rearrange("p (k g) -> p k g", k=KC)
            onescol = pars_sb[:, 444:445]
            selt32 = parsg_sb[:, 0:512].rearrange("g (k c) -> g k c", k=KC)
            selt8 = parsg_sb[0:GC, 1028:1540].rearrange("g (k c) -> g k c", k=KC)
            eps32 = parsg_sb[:, 1024:1025]
            eps8 = parsg_sb[0:GC, 1024:1025]

            # PE warm-up from t~0: matmuls on a Pool-memset scratch tile hold
            # the PE through its p-state ramp so the first real matmuls run at
            # full speed. Results are discarded.
            nc.gpsimd.memset(wmup[:], 0.0)
            # enough back-to-back warm-up matmuls to bridge to the first GN1
            # group matmuls (~8us) -- a >~2us PE idle gap resets the ramp
            for _ in range(26):
                ps_w = qp.tile([P, 512], f32, tag="qp")
                nc.tensor.matmul(
                    ps_w[:, 0:448],
                    wmup[:, 0:128],
                    wmup[:, 0:448],
                    start=True, stop=True,
                )

            # ---------- input DMAs, ordered for the GN1 -> conv chain -------
            # chunk 2 lands first (its stats run on ACT), then chunk 3 so the
            # DVE's last bn_stats isn't the straggler; pair-0 weights ship in
            # their own small contiguous DMA.
            nc.sync.dma_start(x_sb[:, 2, :], x8_d[:, 2, :])
            nc.sync.dma_start(x_sb[:, 3, :], x8_d[:, 3, :])
            nc.sync.dma_start(x_sb[:, 0, :], x8_d[:, 0, :])
            nc.sync.dma_start(x_sb[:, 1, :], x8_d[:, 1, :])
            nc.sync.dma_start(pars_sb[:], pars_d[:])
            nc.sync.dma_start(parsg_sb[:], parsg_d[:])
            wqk_sb = wpool.tile([P, KC, 2, KC, P], f16, tag="wqk")
            nc.sync.dma_start(wqk_sb[:, 0], wqk_d[:, 0])
            nc.sync.dma_start(wqk_sb[:, 1:KC], wqk_d[:, 1:KC])
            wvp_sb = wpool.tile([P, WT], f16, tag="wvp")
            nc.sync.dma_start(wvp_sb[:], wvp_d[:])
            nc.sync.dma_start(out_d[:], x_d[:])

            wq_sb = wqk_sb[:, :, 0]
            wk_sb = wqk_sb[:, :, 1]
            wv_sb = wvp_sb[:, 0 : KC * C].rearrange("p (k c) -> p k c", k=KC)
            wp_sb = wvp_sb[:, KC * C : WI].rearrange("p (k c) -> p k c", k=KC)
            ident_sb = wvp_sb[:, WI : WI + P]
            negid8 = wvp_sb[:, WI + P : WT].bitcast(f8)

            # Preload the combined ln+exp ACT table set once (Ln/Exp are used
            # for the GroupNorm rstd), so the bacc table-load pass doesn't
            # thrash between sets.
            _set_names = list(get_activation_tables(nc.m.arch).keys())
            _tl = mybir.InstLoadActFuncSet(
                name=nc.get_next_instruction_name(),
                ins=[],
                outs=[],
                act_func_set_id=_set_names.index("natural_log_exp_and_others"),
            )
            _tl.engine = mybir.EngineType.Activation
            nc.scalar.add_instruction(_tl)

            nc.gpsimd.tensor_copy(
                out=vt[:, :, :, CH : CH + 1],
                in_=onescol[:, :, None, None].to_broadcast((P, SC, NHEAD, 1)),
            )

            # ---------- per-chunk GroupNorm chain (used by GN2) ----------
            def gn_chunk(k, mvs_ap, gs, gb, ab, bias_fold):
                """ab[:, k] = per-channel (A, B) for y = src*A + B, given
                per-channel (mean, var) in mvs_ap ([P, 2], chunk k); the
                [P, 1] AP bias_fold adjusts the stats and B as if it had been
                added to the source. DVE ops read the group-reduce PSUM
                directly to keep the dependency chain short."""
                stat2 = tmpp.tile([P, 2], f32, tag="stat2")
                nc.vector.tensor_tensor(
                    stat2[:, 0:1], mvs_ap[:, 0:1], bias_fold, OP.add
                )
                musq = tmpp.tile([P, 1], f32, tag="musq")
                nc.vector.tensor_tensor(musq[:], stat2[:, 0:1], stat2[:, 0:1], OP.mult)
                nc.vector.tensor_tensor(stat2[:, 1:2], mvs_ap[:, 1:2], musq[:], OP.add)
                ps_g = apool.tile([P, 260], f32, tag="apool")
                nc.tensor.matmul(
                    ps_g[0:GC, 0:2], sel8[:, k, :], stat2[:, :],
                    start=True, stop=True,
                )
                gstat = tmpp.tile([GC, 2], f32, tag="gstat")
                nc.vector.tensor_copy(out=gstat[:, 0:1], in_=ps_g[0:GC, 0:1])
                gvar = tmpp.tile([GC, 1], f32, tag="gvar")
                gmusq = tmpp.tile([GC, 1], f32, tag="gmusq")
                # only one PSUM operand allowed per instruction: square the
                # SBUF copy of the group mean
                nc.vector.tensor_tensor(gmusq[:], gstat[:, 0:1], gstat[:, 0:1], OP.mult)
                nc.vector.tensor_tensor(gvar[:], ps_g[0:GC, 1:2], gmusq[:], OP.subtract)
                nc.scalar.activation(out=gvar[:], in_=gvar[:], func=AF.Ln, bias=eps8)
                nc.scalar.activation(out=gstat[:, 1:2], in_=gvar[:], func=AF.Exp, scale=-0.5)
                ps_c = apool.tile([P, 260], f32, tag="apool")
                nc.tensor.matmul(
                    ps_c[:, 0:2], selt8[:, k, :], gstat[:],
                    start=True, stop=True,
                )
                nc.vector.tensor_tensor(
                    ab[:, k, 0:1], gs[:, k : k + 1], ps_c[:, 1:2], OP.mult
                )
                # B = gb + A*(bias_fold - mean_c)
                ma = tmpp.tile([P, 2], f32, tag="ma")
                nc.vector.tensor_tensor(ma[:, 0:1], bias_fold, ps_c[:, 0:1], OP.subtract)
                nc.vector.tensor_tensor(ma[:, 1:2], ab[:, k, 0:1], ma[:, 0:1], OP.mult)
                nc.vector.tensor_tensor(
                    ab[:, k, 1:2], gb[:, k : k + 1], ma[:, 1:2], OP.add
                )

            # ---------- GN1 -> hn (stats chase the x chunk DMAs: ACT
            # accumulators for chunk 2 (lands first), DVE bn_stats for chunks
            # 3,0,1 in arrival order; one batched group reduce + affine) -----
            BN_CHUNKS = (3, 0, 1)
            mvs1 = big.tile([P, 3, 2], f32, tag="mvs1")
            stat2 = big.tile([P, KC, 2], f32, tag="stat2b")
            nc.scalar.activation(
                out=scr16[:, 0, :], in_=x_sb[:, 2, :], func=AF.Copy,
                scale=1.0 / N, accum_out=stat2[:, 2, 0:1],
            )
            nc.scalar.activation(
                out=scr16[:, 1, :], in_=x_sb[:, 2, :], func=AF.Square,
                scale=1.0 / 32.0, accum_out=stat2[:, 2, 1:2],
            )
            for i, k in enumerate(BN_CHUNKS):
                stats = tmpp.tile([P, 2, 6], f32, tag="bnstats")
                resh = x_sb[:, k, :].rearrange("p (s f) -> p s f", f=512)
                for si in range(2):
                    nc.vector.bn_stats(out=stats[:, si, :], in_=resh[:, si, :])
                nc.vector.bn_aggr(out=mvs1[:, i, :], in_=stats[:])
            musq = tmpp.tile([P, 3], f32, tag="musqb")
            nc.vector.tensor_tensor(musq[:], mvs1[:, :, 0], mvs1[:, :, 0], OP.mult)
            for i, k in enumerate(BN_CHUNKS):
                nc.vector.tensor_tensor(
                    stat2[:, k, 1:2], mvs1[:, i, 1:2], musq[:, i : i + 1], OP.add
                )
                nc.vector.tensor_copy(out=stat2[:, k, 0:1], in_=mvs1[:, i, 0:1])
            ps_g1 = qp.tile([P, 512], f32, tag="qp")
            for k in range(KC):
                nc.tensor.matmul(
                    ps_g1[0:GROUPS, 0:2], sel32[:, k, :], stat2[:, k, :],
                    start=(k == 0), stop=(k == KC - 1),
                )
            gst1 = big.tile([GROUPS, 2], f32, tag="gst1")
            gms1 = tmpp.tile([GROUPS, 2], f32, tag="gms1")
            nc.vector.tensor_copy(out=gms1[:], in_=ps_g1[0:GROUPS, 0:2])
            nc.vector.tensor_copy(out=gst1[:, 0:1], in_=gms1[:, 0:1])
            gv1 = tmpp.tile([GROUPS, 1], f32, tag="gv1")
            gmu1 = tmpp.tile([GROUPS, 1], f32, tag="gmu1")
            nc.vector.tensor_tensor(gmu1[:], gms1[:, 0:1], gms1[:, 0:1], OP.mult)
            nc.vector.tensor_tensor(gv1[:], gms1[:, 1:2], gmu1[:], OP.subtract)
            nc.scalar.activation(out=gv1[:], in_=gv1[:], func=AF.Ln, bias=eps32)
            nc.scalar.activation(out=gst1[:, 1:2], in_=gv1[:], func=AF.Exp, scale=-0.5)
            ps_c1 = qp.tile([P, 512], f32, tag="qp")
            for k in range(KC):
                nc.tensor.matmul(
                    ps_c1[:, 2 * k : 2 * k + 2], selt32[:, k, :], gst1[:],
                    start=True, stop=True,
                )
            cst1 = tmpp.tile([P, KC, 2], f32, tag="cst1")
            nc.vector.tensor_copy(out=cst1[:], in_=ps_c1[:, 0 : 2 * KC])
            nc.vector.tensor_tensor(ab1[:, :, 0], g1s[:, :], cst1[:, :, 1], OP.mult)
            ma1 = tmpp.tile([P, KC], f32, tag="ma1")
            nc.vector.tensor_tensor(ma1[:], cst1[:, :, 0], ab1[:, :, 0], OP.mult)
            nc.vector.tensor_tensor(ab1[:, :, 1], g1b[:, :], ma1[:], OP.subtract)
            for k in range(KC):
                nc.vector.tensor_scalar(
                    hn[:, k, :], x_sb[:, k, :],
                    ab1[:, k, 0:1], ab1[:, k, 1:2], OP.mult, OP.add,
                )

            # ---------- phase helpers ----------
            def evac(eng, dst, src, bias_ap):
                if eng is None:
                    # ACT bias-add copy (idle pre-stream)
                    nc.scalar.activation(
                        out=dst, in_=src, func=AF.Identity, bias=bias_ap
                    )
                else:
                    eng.tensor_scalar(dst, src, bias_ap, None, OP.add)

            def conv_q(p, qt, eng):
                # q as hi+lo fp8 pair: evac hi, subtract rne8(hi) from the
                # PSUM via an accumulated -I @ hi matmul, evac the residual
                # on the (otherwise idle) Pool so the DVE keeps up with the
                # Schraudolph share of the exp stream.
                bias_ap = bq_sb[:, p : p + 1]
                for t in range(NT):
                    ts_ = slice(t * 512, (t + 1) * 512)
                    ps = qp.tile([P, 512], f32, tag="qp")
                    for k in range(KC):
                        nc.tensor.matmul(
                            ps[:, :],
                            wq_sb[:, p, k, :],
                            hn[:, k, ts_],
                            start=(k == 0), stop=(k == KC - 1),
                        )
                    evac(eng, qt[:, 0, ts_], ps[:], bias_ap)
                    nc.tensor.matmul(
                        ps[:, :], negid8[:], qt[:, 0, ts_],
                        start=False, stop=True, skip_group_check=True,
                    )
                    # pair 0's lo goes to the DVE (free once hn is done) so
                    # the ACT-side hi evacs and the first exps aren't queued
                    # behind it
                    evac(nc.vector if eng is None else eng,
                         qt[:, 1, ts_], ps[:], bias_ap)

            def conv_k(p, kt, eng, trange=range(NT)):
                bias_ap = bk_sb[:, p : p + 1]
                for t in trange:
                    ts_ = slice(t * 512, (t + 1) * 512)
                    ps = qp.tile([P, 512], f32, tag="qp")
                    for k in range(KC):
                        nc.tensor.matmul(
                            ps[:, :],
                            wk_sb[:, p, k, :],
                            hn[:, k, ts_],
                            start=(k == 0), stop=(k == KC - 1),
                        )
                    evac(eng, kt[:, 0, ts_], ps[:], bias_ap)
                    # duplicate into subtile 1 for the DoubleRow layout
                    nc.gpsimd.tensor_copy(out=kt[:, 1, ts_], in_=kt[:, 0, ts_])

            def v_tiles(half):
                # v^T tiles [s-part, head-major channel]; bv is folded into
                # the proj bias on the host, so no bias row here. Built in
                # halves slotted into heads 0 and 1 so the DVE evacuation
                # copies don't pile up in one stream window.
                for nt in range(4 * half, 4 * half + 4):
                    ps = qp.tile([P, 512], f32, tag="qp")
                    for k in range(KC):
                        nc.tensor.matmul(
                            ps[:, :],
                            hn[:, k, nt * P : (nt + 1) * P],
                            wv_sb[:, k, :],
                            start=(k == 0), stop=(k == KC - 1),
                        )
                    nc.vector.tensor_copy(
                        out=vt[:, nt, :, 0:CH],
                        in_=ps[:, :].rearrange("p (h c) -> p h c", h=NHEAD),
                    )

            # s-chunks routed to the DVE via the Schraudolph 2^x bit trick:
            # i16 = round(S*1024*log2(e) + (15*1024 - 62.2)) reinterpreted as
            # fp16 approximates exp(S) to ~+-4% -- the softmax denominator
            # uses the same approximated values, so the common mode cancels.
            SCHRA = (2, 5)
            SC1 = 1024 * 1.4426950408889634
            SC2 = 15360.0 - 62.2

            def head_st(h, qt, kt, mid=None):
                # S^T as fp8 DoubleRow: subtiles = (q_hi, q_lo) against a
                # duplicated k8, so each [128, 512] output costs 256 PE
                # cycles at near-fp16 accuracy.
                p, e = h // 2, h % 2
                rows = slice(64 * e, 64 * e + 64)
                exps = expp.tile([P, SC, N], f16, tag="exps")
                for sc in range(SC):
                    if sc == 2 and mid is not None:
                        mid()
                    if sc in SCHRA:
                        # Schraudolph tiles go through qp halves so the
                        # ACT-fed stp pipeline never waits on the DVE
                        for t in range(NT):
                            pq = qp.tile([P, 512], f32, tag="qp")
                            nc.tensor.matmul(
                                pq[:, :],
                                kt[rows, :, sc * P : (sc + 1) * P],
                                qt[rows, :, t * 512 : (t + 1) * 512],
                                start=True, stop=True,
                                perf_mode=PM.DoubleRow,
                            )
                            nc.vector.tensor_scalar(
                                exps[:, sc, t * 512 : (t + 1) * 512].bitcast(i16),
                                pq[:], SC1, SC2, OP.mult, OP.add,
                            )
                        continue
                    ps_st = stp.tile([P, N], f32, tag="stp")
                    for t in range(NT):
                        nc.tensor.matmul(
                            ps_st[:, t * 512 : (t + 1) * 512],
                            kt[rows, :, sc * P : (sc + 1) * P],
                            qt[rows, :, t * 512 : (t + 1) * 512],
                            start=True, stop=True,
                            perf_mode=PM.DoubleRow,
                        )
                    nc.scalar.activation(
                        out=exps[:, sc, :], in_=ps_st[:], func=AF.Exp
                    )
                return exps

            def head_a(h, exps, auT):
                # Transposed A: out a^T[t-part, ch] per 128-wide t-chunk, the
                # softmax denominator lands in column 64 as a per-partition
                # scalar -> one reciprocal + a fused normalize-evacuate.
                # (NOTE: the s accumulation must NOT interleave j groups --
                # PSUM allows one pending accumulation group per bank.)
                e = h % 2
                for u in range(2):
                    ps_aT = apool.tile([P, 260], f32, tag="apool")
                    pv = ps_aT[:].rearrange("p (j c) -> p j c", c=65)
                    for j in range(4):
                        tch = 4 * u + j
                        for sc in range(SC):
                            nc.tensor.matmul(
                                pv[:, j, :],
                                exps[:, sc, tch * P : (tch + 1) * P],
                                vt[:, sc, h, :],
                                start=(sc == 0), stop=(sc == SC - 1),
                            )
                    rcol = tmpp.tile([P, 4], f32, tag="rcol")
                    nc.vector.reciprocal(rcol[:], pv[:, :, 64])
                    for j in range(4):
                        tch = 4 * u + j
                        nc.vector.tensor_scalar(
                            auT[:, tch, 64 * e : 64 * e + 64], pv[:, j, 0:64],
                            rcol[:, j : j + 1], None, OP.mult,
                        )

            # ---------- qkv, then attention ----------
            def conv_pair(p, eng):
                qt = qpool.tile([P, 2, N], f8, tag="qt")
                kt = kpool.tile([P, 2, N], f8, tag="kt")
                conv_k(p, kt, eng, trange=(0,))
                conv_q(p, qt, eng)
                conv_k(p, kt, eng, trange=(1,))
                return qt, kt

            # pair 0 evacuates on the still-idle ACT so the stream starts
            # as early as possible; later pairs use the DVE.
            qt, kt = conv_pair(0, None)
            e = {}
            e[0] = head_st(0, qt, kt, mid=lambda: v_tiles(0))
            # each later conv pair is emitted a full head EARLY so its DVE
            # evacuations drain a head-window before the S^T that needs them
            # (they otherwise collide with that window's Schraudolph +
            # normalize work and stall the exp stream)
            nqt, nkt = conv_pair(1, nc.vector)
            # v^T tiles build on the PE while head 0/1's exps stream, slotted
            # into the middles of both heads' S^T so exp starts on time
            e[1] = head_st(1, qt, kt, mid=lambda: v_tiles(1))
            auTs = {}

            def transposes(p):
                # chunks 0/1 ride the XBAR mid-stream; chunks 2/3 take PE
                # identity-transposes -- the XBAR's HWDGE serialization
                # (~5us for 8 tiles + ~1us completion sem) would gate the
                # proj pre-runs right at the stream tail
                if p < KC - 2:
                    # XBAR transpose a^T -> a_u chunk p (SBUF->SBUF, no
                    # PE/DVE time)
                    for tch in range(SC):
                        nc.sync.dma_start_transpose(
                            a_u[:, p, tch * P : (tch + 1) * P], auTs[p][:, tch, :]
                        )
                else:
                    # tail chunk: PE transposes (identity matmul) -> shortest
                    # path into proj's k=3 contraction; the PSUM->SBUF copies
                    # run on the post-stream-idle ACT so the DVE (busy with
                    # the last head's normalize) isn't the serializer
                    for tch in range(SC):
                        ps_t = apool.tile([P, 260], f32, tag="apool")
                        pt = ps_t[:].bitcast(f16)
                        nc.tensor.matmul(
                            pt[:, 0:P], auTs[p][:, tch, :], ident_sb[:],
                            is_transpose=True,
                        )
                        if p == KC - 1:
                            # post-stream: ACT is idle, keep the DVE free for
                            # the bn/gn chains
                            nc.scalar.activation(
                                out=a_u[:, p, tch * P : (tch + 1) * P],
                                in_=pt[:, 0:P], func=AF.Copy,
                            )
                        else:
                            nc.vector.tensor_copy(
                                out=a_u[:, p, tch * P : (tch + 1) * P],
                                in_=pt[:, 0:P],
                            )

            # software pipeline: S^T/exp of pair p streams while the A
            # matmuls of pair p-1 drain, so pair boundaries stay dense
            for p in range(1, KC):
                qt, kt = nqt, nkt
                h0 = 2 * (p - 1)
                e[2 * p] = head_st(2 * p, qt, kt)
                if p < KC - 1:
                    nqt, nkt = conv_pair(p + 1, nc.vector)
                auT = autp.tile([P, SC, P], f16, tag="auT")
                auTs[p - 1] = auT
                head_a(h0, e[h0], auTs[p - 1])
                e[2 * p + 1] = head_st(2 * p + 1, qt, kt)
                head_a(h0 + 1, e[h0 + 1], auTs[p - 1])
                transposes(p - 1)
            auT = autp.tile([P, SC, P], f16, tag="auT")
            auTs[KC - 1] = auT
            head_a(6, e[6], auTs[KC - 1])

            head_a(7, e[7], auTs[KC - 1])

            # proj m0's k<3 accumulation also pre-runs under the stream tail
            preruns = {}
            ps_pm0 = stp.tile([P, N], f32, tag="stp")
            preruns[0] = ps_pm0
            for t in range(NT):
                for k in range(KC - 1):
                    nc.tensor.matmul(
                        preruns[0][:, t * 512 : (t + 1) * 512],
                        wp_sb[:, k, 0:P],
                        a_u[:, k, t * 512 : (t + 1) * 512],
                        start=(k == 0), stop=False,
                        skip_group_check=True,
                    )

            transposes(KC - 1)

            # ---------- proj + GN2 + residual, pipelined per chunk ----------
            # DVE bn_stats reads the proj PSUM directly (one pass for mean
            # and var); the per-chunk group reduce + affine then run on the
            # post-stream-idle PE/ACT, and one ACT Identity applies
            # y = ps*A + B straight out of PSUM. The (host-folded) proj bias
            # enters the stats and the B term via gn_chunk's bias_fold.
            mvs2 = big.tile([P, KC, 2], f32, tag="mvs2")
            # m0 and m1 finish their pre-run accumulators and take their
            # stats back-to-back BEFORE either gn chain runs, so the second
            # bn pass isn't stuck behind the first gn's PE/ACT hops in the
            # in-order DVE queue
            pr_halves = {}
            for m in preruns:
                halves = [
                    preruns[m][:, t * 512 : (t + 1) * 512] for t in range(NT)
                ]
                pr_halves[m] = halves
                stats = tmpp.tile([P, 2, 6], f32, tag="bnstats")
                for t in range(NT):
                    nc.tensor.matmul(
                        halves[t],
                        wp_sb[:, KC - 1, m * P : (m + 1) * P],
                        a_u[:, KC - 1, t * 512 : (t + 1) * 512],
                        start=False, stop=True,
                        skip_group_check=True,
                    )
                    nc.vector.bn_stats(out=stats[:, t, :], in_=halves[t])
                nc.vector.bn_aggr(out=mvs2[:, m, :], in_=stats[:])
            for m in range(KC):
                # m2 goes through two qp half-banks so it never waits on an
                # earlier chunk's apply to free stp; m3 recycles the first
                # freed stp buffer.
                if m in preruns:
                    halves = pr_halves[m]
                elif m == 2:
                    halves = []
                    for _t in range(NT):
                        psh = qp.tile([P, 512], f32, tag="qp")
                        halves.append(psh[:])
                else:
                    psm = stp.tile([P, N], f32, tag="stp")
                    halves = [psm[:, t * 512 : (t + 1) * 512] for t in range(NT)]
                if m not in preruns:
                    stats = tmpp.tile([P, 2, 6], f32, tag="bnstats")
                    for t in range(NT):
                        for k in range(KC):
                            nc.tensor.matmul(
                                halves[t],
                                wp_sb[:, k, m * P : (m + 1) * P],
                                a_u[:, k, t * 512 : (t + 1) * 512],
                                start=(k == 0),
                                stop=(k == KC - 1),
                                skip_group_check=True,
                            )
                        # stats on the finished half while the other runs
                        nc.vector.bn_stats(out=stats[:, t, :], in_=halves[t])
                    nc.vector.bn_aggr(out=mvs2[:, m, :], in_=stats[:])
                gn_chunk(m, mvs2[:, m, :], g2s, g2b, ab2,
                         bias_fold=bp_sb[:, m : m + 1])
                for t in range(NT):
                    nc.scalar.activation(
                        out=projf[:, m, t * 512 : (t + 1) * 512], in_=halves[t],
                        func=AF.Identity,
                        scale=ab2[:, m, 0:1], bias=ab2[:, m, 1:2],
                    )
                for t in range(NT):
                    nc.gpsimd.dma_start(
                        out_d[:, m, t * 512 : (t + 1) * 512],
                        projf[:, m, t * 512 : (t + 1) * 512],
                        accum_op=OP.add,
                    )

    nc.compile()
    return nc


def _host_prep(x, gn1_scale, gn1_bias, w_qkv, b_qkv, w_proj, b_proj, gn2_scale, gn2_bias):
    """Build per-core input maps (numpy only)."""
    f = np.float32
    bf = np.float16
    x = np.asarray(x, f)
    w_qkv = np.asarray(w_qkv, f)
    b_qkv = np.asarray(b_qkv, f)
    w_proj = np.asarray(w_proj, f)
    b_proj = np.asarray(b_proj, f)
    gn1_scale = np.asarray(gn1_scale, f)
    gn1_bias = np.asarray(gn1_bias, f)
    gn2_scale = np.asarray(gn2_scale, f)
    gn2_bias = np.asarray(gn2_bias, f)

    def chunk_vec(v):  # [C] -> [P, KC]
        return np.ascontiguousarray(v.reshape(KC, P).T)

    def chunk_mat(wt, dt=f):  # [C, M] -> [P, KC, M]
        return np.ascontiguousarray(wt.reshape(KC, P, -1).transpose(1, 0, 2).astype(dt))

    idx = np.arange(NHEAD)[:, None] * (3 * CH) + np.arange(CH)[None, :]
    q_idx, k_idx, v_idx = idx.ravel(), (idx + CH).ravel(), (idx + 2 * CH).ravel()

    s1 = float(CH) ** -0.25
    def mtile(w):  # [P, KC, C] -> [P, M, KC, P]
        return np.ascontiguousarray(
            w.reshape(P, KC, KC, P).transpose(0, 2, 1, 3)
        )
    wq = mtile(chunk_mat(w_qkv[q_idx].T * s1, bf))
    wk = mtile(chunk_mat(w_qkv[k_idx].T * s1, bf))
    wv = chunk_mat(w_qkv[v_idx].T, bf)
    wp = chunk_mat(w_proj.T, bf)
    wqk = np.ascontiguousarray(np.stack([wq, wk], axis=2))

    WI = 2 * KC * C
    WT = WI + P + P // 2
    wvp = np.zeros((P, WT), bf)
    wvp[:, 0 : KC * C] = wv.reshape(P, -1)
    wvp[:, KC * C : WI] = wp.reshape(P, -1)
    wvp[:, WI : WI + P] = np.eye(P, dtype=bf)
    # fp8e4m3 -I bit-packed into f16 lanes: -1.0 is 0xB8
    ni = np.zeros((P, P), np.uint8)
    ni[np.arange(P), np.arange(P)] = 0xB8
    wvp[:, WI + P : WT] = ni.view(np.uint16).view(bf)

    bq = chunk_vec(b_qkv[q_idx] * s1)
    bk = chunk_vec(b_qkv[k_idx] * s1)
    # v-bias folded into the proj bias: proj(a + bv) = proj(a) + w_proj @ bv
    bp_eff = b_proj + w_proj @ b_qkv[v_idx]
    bp = chunk_vec(bp_eff)

    cidx = np.arange(C)
    sel8 = np.zeros((P, KC, GC), f)
    sel8[cidx % P, cidx // P, (cidx % P) // 16] = 1.0 / 16.0
    selt8 = np.zeros((GC, KC, P), f)
    selt8[(cidx % P) // 16, cidx // P, cidx % P] = 1.0
    sel32 = np.zeros((P, KC, GROUPS), f)
    sel32[cidx % P, cidx // P, cidx // 16] = 1.0 / 16.0
    selt32 = np.zeros((GROUPS, KC, P), f)
    selt32[cidx // 16, cidx // P, cidx % P] = 1.0

    pars = np.zeros((P, 448), f)
    pars[:, 0:4] = chunk_vec(gn1_scale)
    pars[:, 4:8] = chunk_vec(gn1_bias)
    pars[:, 8:12] = bq
    pars[:, 12:16] = bk
    pars[:, 16:20] = bp
    pars[:, 20:24] = chunk_vec(gn2_scale)
    pars[:, 24:28] = chunk_vec(gn2_bias)
    pars[:, 28:60] = sel8.reshape(P, -1)
    pars[:, 60:188] = sel32.reshape(P, -1)
    pars[:, 444] = 1.0
    parsg = np.zeros((GROUPS, 1544), f)
    parsg[:, 0:512] = selt32.reshape(GROUPS, -1)
    parsg[:, 1024] = EPS
    parsg[0:GC, 1028:1540] = selt8.reshape(GC, -1)

    shared = {
        "wqk": wqk, "wvp": wvp, "pars": pars, "parsg": parsg,
    }
    in_maps = []
    for b in range(B):
        xb = np.ascontiguousarray(
            x[b].reshape(C, N).reshape(KC, P, N).transpose(1, 0, 2)
        )
        in_maps.append({"x": xb, "x8": xb.astype(bf), **shared})
    return in_maps


def _assemble(results):
    out = np.empty((B, C, H, W), np.float32)
    for b in range(B):
        ob = np.asarray(results[b]["out"])  # [P, KC, N]
        out[b] = ob.transpose(1, 0, 2).reshape(C, N).reshape(C, H, W)
    return out


def get_nc():
    if "nc" not in _CACHE:
        _CACHE["nc"] = _build_nc()
    return _CACHE["nc"]


def kernel(x, gn1_scale, gn1_bias, w_qkv, b_qkv, w_proj, b_proj, gn2_scale, gn2_bias):
    from concourse.bass_utils import run_bass_kernel_spmd

    nc = get_nc()
    in_maps = _host_prep(
        x, gn1_scale, gn1_bias, w_qkv, b_qkv, w_proj, b_proj, gn2_scale, gn2_bias
    )
    res = run_bass_kernel_spmd(nc, in_maps, core_ids=list(range(B)))
    return _assemble(res.results)


# revision 68
# speedup vs baseline: 1.0215x; 1.0114x over previous
"""Attention2d Trainium2 Bass kernel.

Reference computation (per batch element b of 8, one NeuronCore each):
    hn  = GroupNorm32(x) * gn1_scale + gn1_bias
    qkv = w_qkv @ hn + b_qkv          (1x1 conv == matmul over channels)
    per head h (8 heads, ch=64): q,k,v from qkv (torch reshape convention:
        head h uses rows h*192+{0..64,64..128,128..192})
    wgt = softmax((q*s)^T (k*s)), s = ch**-0.25
    a   = v @ wgt^T
    out = GroupNorm32(w_proj @ a + b_proj) ... * gn2_scale + gn2_bias
    y   = x + out

Device strategy (data-parallel over batch, 1 core per batch element):
  - ACT is the critical engine: the 8 heads' exp(S^T) stream is 64 tiles of
    [128, 1024] (~1.04us each). Everything else is scheduled so that stream
    never starves: PE work is held well under the stream duration.
  - S^T runs as fp8e4 DoubleRow matmuls at 0.5 cycles/row. To keep the
    quantization error inside the rel-err budget, q ships as TWO fp8
    k-subtiles (hi + residual lo, together fp16-accurate) against a
    DUPLICATED fp8 k: S = k8^T(q_hi + q_lo), so only k's single fp8
    rounding touches the logits. q_lo is formed by accumulating -I @ q_hi
    into the conv PSUM (one cheap fp8 matmul) and re-evacuating.
  - exps, v^T tiles, the A matmuls, convs and proj all stay fp16: their
    quantization hits the output directly (measured: fp8 exps alone cost
    2.6e-2 rel err), while the k-side fp8 is dampened through softmax.
  - the A matmul runs TRANSPOSED: out a^T[t-part, ch] per 128-wide t-chunk;
    the softmax denominator (ones-column of v^T) lands in column 64 as a
    per-partition scalar: one DVE reciprocal + one fused tensor_scalar
    normalizes while evacuating. The LAST head's A accumulation is split
    into s-halves so only half of it trails the final exp tile.
  - a^T -> a via XBAR dma_start_transpose (no engine time); the LAST pair
    uses PE identity-transposes so proj isn't tail-gated by the HWDGE queue
  - v-bias folded into the proj bias on the host; proj bias folded into the
    GN2 affine + channel stats
  - GN2: DVE bn_stats reads each proj PSUM tile directly (one pass), the
    group reduce + affine run on PE/ACT (idle post-stream), and a single
    ACT Identity applies y = ps*A + B straight out of PSUM
  - input DMAs are packed and ordered so GN1 stats chase the x chunks
    (ACT takes the chunk that lands first, DVE the rest), pair-0 weights
    ship in their own small DMA, and pair-0's q/k evacuations run on the
    still-idle ACT so the first exp fires as early as possible
  - scratch warm-up matmuls on a memset tile hold the PE in its fast
    p-state from t~0
"""

import numpy as np

NHEAD = 8
GROUPS = 32
EPS = 1e-5
B, C, H, W = 8, 512, 32, 32
N = H * W            # 1024 spatial positions
CH = C // NHEAD      # 64 channels per head
P = 128              # partitions
KC = C // P          # 4 channel chunks
NT = N // 512        # 2 column tiles of 512
SC = N // P          # 8 s-chunks
GC = GROUPS // KC    # 8 groups per chunk

_CACHE = {}


def _build_nc():
    import concourse.tile as tile
    from concourse import mybir, bacc
    from concourse.hw_specs import get_activation_tables

    f32 = mybir.dt.float32
    f16 = mybir.dt.float16
    f8 = mybir.dt.float8e4
    i16 = mybir.dt.int16
    AF = mybir.ActivationFunctionType
    OP = mybir.AluOpType
    PM = mybir.MatmulPerfMode

    nc = bacc.Bacc("TRN2", target_bir_lowering=False, num_devices=8)

    WI = 2 * KC * C          # wv/wp block columns in the packed weight tile
    WT = WI + P + P // 2     # + f16 identity + fp8 -identity (bit-packed)

    x_d = nc.dram_tensor("x", [P, KC, N], f32, kind="ExternalInput")
    x8_d = nc.dram_tensor("x8", [P, KC, N], f16, kind="ExternalInput")
    wqk_d = nc.dram_tensor("wqk", [P, KC, 2, KC, P], f16, kind="ExternalInput")
    wvp_d = nc.dram_tensor("wvp", [P, WT], f16, kind="ExternalInput")
    pars_d = nc.dram_tensor("pars", [P, 448], f32, kind="ExternalInput")
    parsg_d = nc.dram_tensor("parsg", [GROUPS, 1544], f32, kind="ExternalInput")
    out_d = nc.dram_tensor("out", [P, KC, N], f32, kind="ExternalOutput")

    with tile.TileContext(nc) as tc:
        with (
            tc.tile_pool(name="big", bufs=1) as big,
            tc.tile_pool(name="wpool", bufs=2) as wpool,
            tc.tile_pool(name="qpool", bufs=2) as qpool,
            tc.tile_pool(name="kpool", bufs=4) as kpool,
            tc.tile_pool(name="vtp", bufs=1) as vtp,
            tc.tile_pool(name="expp", bufs=3) as expp,
            tc.tile_pool(name="autp", bufs=2) as autp,
            tc.tile_pool(name="tmpp", bufs=2) as tmpp,
            tc.tile_pool(name="stp", bufs=2, space="PSUM") as stp,
            tc.tile_pool(name="apool", bufs=2, space="PSUM") as apool,
            tc.tile_pool(name="qp", bufs=2, space="PSUM") as qp,
        ):
            # ---------- persistent SBUF tiles ----------
            x_sb = big.tile([P, KC, N], f16, tag="x_sb")
            hn = big.tile([P, KC, N], f16, tag="hn")
            vt = vtp.tile([P, SC, NHEAD, CH + 1], f16, tag="vp")
            pars_sb = big.tile([P, 448], f32, tag="pars_sb")
            parsg_sb = big.tile([GROUPS, 1544], f32, tag="parsg_sb")
            ab1 = big.tile([P, KC, 2], f32, tag="ab1")
            ab2 = big.tile([P, KC, 2], f32, tag="ab2")
            projf = big.tile([P, KC, N], f32, tag="projf")
            a_u = big.tile([P, KC, N], f16, tag="a_u")
            scr16 = big.tile([P, 2, N], f16, tag="scr16")
            wmup = big.tile([P, 512], f16, tag="wmup")

            # packed-parameter views
            g1s = pars_sb[:, 0:4]
            g1b = pars_sb[:, 4:8]
            bq_sb = pars_sb[:, 8:12]
            bk_sb = pars_sb[:, 12:16]
            bp_sb = pars_sb[:, 16:20]
            g2s = pars_sb[:, 20:24]
            g2b = pars_sb[:, 24:28]
            sel8 = pars_sb[:, 28:60].rearrange("p (k g) -> p k g", k=KC)
            sel32 = pars_sb[:, 60:188].rearrange("p (k g) -> p k g", k=KC)
            onescol = pars_sb[:, 444:445]
            selt32 = parsg_sb[:, 0:512].rearrange("g (k c) -> g k c", k=KC)
            selt8 = parsg_sb[0:GC, 1028:1540].rearrange("g (k c) -> g k c", k=KC)
            eps32 = parsg_sb[:, 1024:1025]
            eps8 = parsg_sb[0:GC, 1024:1025]

            # PE warm-up from t~0: matmuls on a Pool-memset scratch tile hold
            # the PE through its p-state ramp so the first real matmuls run at
            # full speed. Results are discarded.
            nc.gpsimd.memset(wmup[:], 0.0)
            # enough back-to-back warm-up matmuls to bridge to the first GN1
            # group matmuls (~8us) -- a >~2us PE idle gap resets the ramp
            for _ in range(26):
                ps_w = qp.tile([P, 512], f32, tag="qp")
                nc.tensor.matmul(
                    ps_w[:, 0:448],
                    wmup[:, 0:128],
                    wmup[:, 0:448],
                    start=True, stop=True,
                )

            # ---------- input DMAs, ordered for the GN1 -> conv chain -------
            # chunk 2 lands first (its stats run on ACT), then chunk 3 so the
            # DVE's last bn_stats isn't the straggler; pair-0 weights ship in
            # their own small contiguous DMA.
            nc.sync.dma_start(x_sb[:, 2, :], x8_d[:, 2, :])
            nc.sync.dma_start(x_sb[:, 3, :], x8_d[:, 3, :])
            nc.sync.dma_start(x_sb[:, 0, :], x8_d[:, 0, :])
            nc.sync.dma_start(x_sb[:, 1, :], x8_d[:, 1, :])
            nc.sync.dma_start(pars_sb[:], pars_d[:])
            nc.sync.dma_start(parsg_sb[:], parsg_d[:])
            wqk_sb = wpool.tile([P, KC, 2, KC, P], f16, tag="wqk")
            nc.sync.dma_start(wqk_sb[:, 0], wqk_d[:, 0])
            nc.sync.dma_start(wqk_sb[:, 1:KC], wqk_d[:, 1:KC])
            wvp_sb = wpool.tile([P, WT], f16, tag="wvp")
            nc.sync.dma_start(wvp_sb[:], wvp_d[:])
            nc.sync.dma_start(out_d[:], x_d[:])

            wq_sb = wqk_sb[:, :, 0]
            wk_sb = wqk_sb[:, :, 1]
            wv_sb = wvp_sb[:, 0 : KC * C].rearrange("p (k c) -> p k c", k=KC)
            wp_sb = wvp_sb[:, KC * C : WI].rearrange("p (k c) -> p k c", k=KC)
            ident_sb = wvp_sb[:, WI : WI + P]
            negid8 = wvp_sb[:, WI + P : WT].bitcast(f8)

            # Preload the combined ln+exp ACT table set once (Ln/Exp are used
            # for the GroupNorm rstd), so the bacc table-load pass doesn't
            # thrash between sets.
            _set_names = list(get_activation_tables(nc.m.arch).keys())
            _tl = mybir.InstLoadActFuncSet(
                name=nc.get_next_instruction_name(),
                ins=[],
                outs=[],
                act_func_set_id=_set_names.index("natural_log_exp_and_others"),
            )
            _tl.engine = mybir.EngineType.Activation
            nc.scalar.add_instruction(_tl)

            nc.gpsimd.tensor_copy(
                out=vt[:, :, :, CH : CH + 1],
                in_=onescol[:, :, None, None].to_broadcast((P, SC, NHEAD, 1)),
            )

            # ---------- per-chunk GroupNorm chain (used by GN2) ----------
            def gn_chunk(k, mvs_ap, gs, gb, ab, bias_fold):
                """ab[:, k] = per-channel (A, B) for y = src*A + B, given
                per-channel (mean, var) in mvs_ap ([P, 2], chunk k); the
                [P, 1] AP bias_fold adjusts the stats and B as if it had been
                added to the source. DVE ops read the group-reduce PSUM
                directly to keep the dependency chain short."""
                stat2 = tmpp.tile([P, 2], f32, tag="stat2")
                nc.vector.tensor_tensor(
                    stat2[:, 0:1], mvs_ap[:, 0:1], bias_fold, OP.add
                )
                musq = tmpp.tile([P, 1], f32, tag="musq")
                nc.vector.tensor_tensor(musq[:], stat2[:, 0:1], stat2[:, 0:1], OP.mult)
                nc.vector.tensor_tensor(stat2[:, 1:2], mvs_ap[:, 1:2], musq[:], OP.add)
                ps_g = apool.tile([P, 260], f32, tag="apool")
                nc.tensor.matmul(
                    ps_g[0:GC, 0:2], sel8[:, k, :], stat2[:, :],
                    start=True, stop=True,
                )
                gstat = tmpp.tile([GC, 2], f32, tag="gstat")
                nc.vector.tensor_copy(out=gstat[:, 0:1], in_=ps_g[0:GC, 0:1])
                gvar = tmpp.tile([GC, 1], f32, tag="gvar")
                gmusq = tmpp.tile([GC, 1], f32, tag="gmusq")
                # only one PSUM operand allowed per instruction: square the
                # SBUF copy of the group mean
                nc.vector.tensor_tensor(gmusq[:], gstat[:, 0:1], gstat[:, 0:1], OP.mult)
                nc.vector.tensor_tensor(gvar[:], ps_g[0:GC, 1:2], gmusq[:], OP.subtract)
                nc.scalar.activation(out=gvar[:], in_=gvar[:], func=AF.Ln, bias=eps8)
                nc.scalar.activation(out=gstat[:, 1:2], in_=gvar[:], func=AF.Exp, scale=-0.5)
                ps_c = apool.tile([P, 260], f32, tag="apool")
                nc.tensor.matmul(
                    ps_c[:, 0:2], selt8[:, k, :], gstat[:],
                    start=True, stop=True,
                )
                nc.vector.tensor_tensor(
                    ab[:, k, 0:1], gs[:, k : k + 1], ps_c[:, 1:2], OP.mult
                )
                # B = gb + A*(bias_fold - mean_c)
                ma = tmpp.tile([P, 2], f32, tag="ma")
                nc.vector.tensor_tensor(ma[:, 0:1], bias_fold, ps_c[:, 0:1], OP.subtract)
                nc.vector.tensor_tensor(ma[:, 1:2], ab[:, k, 0:1], ma[:, 0:1], OP.mult)
                nc.vector.tensor_tensor(
                    ab[:, k, 1:2], gb[:, k : k + 1], ma[:, 1:2], OP.add
                )

            # ---------- GN1 -> hn (stats chase the x chunk DMAs: ACT
            # accumulators for chunk 2 (lands first), DVE bn_stats for chunks
            # 3,0,1 in arrival order; one batched group reduce + affine) -----
            BN_CHUNKS = (3, 0, 1)
            mvs1 = big.tile([P, 3, 2], f32, tag="mvs1")
            stat2 = big.tile([P, KC, 2], f32, tag="stat2b")
            nc.scalar.activation(
                out=scr16[:, 0, :], in_=x_sb[:, 2, :], func=AF.Copy,
                scale=1.0 / N, accum_out=stat2[:, 2, 0:1],
            )
            nc.scalar.activation(
                out=scr16[:, 1, :], in_=x_sb[:, 2, :], func=AF.Square,
                scale=1.0 / 32.0, accum_out=stat2[:, 2, 1:2],
            )
            for i, k in enumerate(BN_CHUNKS):
                stats = tmpp.tile([P, 2, 6], f32, tag="bnstats")
                resh = x_sb[:, k, :].rearrange("p (s f) -> p s f", f=512)
                for si in range(2):
                    nc.vector.bn_stats(out=stats[:, si, :], in_=resh[:, si, :])
                nc.vector.bn_aggr(out=mvs1[:, i, :], in_=stats[:])
            musq = tmpp.tile([P, 3], f32, tag="musqb")
            nc.vector.tensor_tensor(musq[:], mvs1[:, :, 0], mvs1[:, :, 0], OP.mult)
            for i, k in enumerate(BN_CHUNKS):
                nc.vector.tensor_tensor(
                    stat2[:, k, 1:2], mvs1[:, i, 1:2], musq[:, i : i + 1], OP.add
                )
                nc.vector.tensor_copy(out=stat2[:, k, 0:1], in_=mvs1[:, i, 0:1])
            ps_g1 = qp.tile([P, 512], f32, tag="qp")
            for k in range(KC):
                nc.tensor.matmul(
                    ps_g1[0:GROUPS, 0:2], sel32[:, k, :], stat2[:, k, :],
                    start=(k == 0), stop=(k == KC - 1),
                )
            gst1 = big.tile([GROUPS, 2], f32, tag="gst1")
            gms1 = tmpp.tile([GROUPS, 2], f32, tag="gms1")
            nc.vector.tensor_copy(out=gms1[:], in_=ps_g1[0:GROUPS, 0:2])
            nc.vector.tensor_copy(out=gst1[:, 0:1], in_=gms1[:, 0:1])
            gv1 = tmpp.tile([GROUPS, 1], f32, tag="gv1")
            gmu1 = tmpp.tile([GROUPS, 1], f32, tag="gmu1")
            nc.vector.tensor_tensor(gmu1[:], gms1[:, 0:1], gms1[:, 0:1], OP.mult)
            nc.vector.tensor_tensor(gv1[:], gms1[:, 1:2], gmu1[:], OP.subtract)
            nc.scalar.activation(out=gv1[:], in_=gv1[:], func=AF.Ln, bias=eps32)
            nc.scalar.activation(out=gst1[:, 1:2], in_=gv1[:], func=AF.Exp, scale=-0.5)
            ps_c1 = qp.tile([P, 512], f32, tag="qp")
            for k in range(KC):
                nc.tensor.matmul(
                    ps_c1[:, 2 * k : 2 * k + 2], selt32[:, k, :], gst1[:],
                    start=True, stop=True,
                )
            cst1 = tmpp.tile([P, KC, 2], f32, tag="cst1")
            nc.vector.tensor_copy(out=cst1[:], in_=ps_c1[:, 0 : 2 * KC])
            nc.vector.tensor_tensor(ab1[:, :, 0], g1s[:, :], cst1[:, :, 1], OP.mult)
            ma1 = tmpp.tile([P, KC], f32, tag="ma1")
            nc.vector.tensor_tensor(ma1[:], cst1[:, :, 0], ab1[:, :, 0], OP.mult)
            nc.vector.tensor_tensor(ab1[:, :, 1], g1b[:, :], ma1[:], OP.subtract)
            for k in range(KC):
                nc.vector.tensor_scalar(
                    hn[:, k, :], x_sb[:, k, :],
                    ab1[:, k, 0:1], ab1[:, k, 1:2], OP.mult, OP.add,
                )

            # ---------- phase helpers ----------
            def evac(eng, dst, src, bias_ap):
                if eng is None:
                    # ACT bias-add copy (idle pre-stream)
                    nc.scalar.activation(
                        out=dst, in_=src, func=AF.Identity, bias=bias_ap
                    )
                else:
                    eng.tensor_scalar(dst, src, bias_ap, None, OP.add)

            def conv_q(p, qt, eng):
                # q as hi+lo fp8 pair: evac hi, subtract rne8(hi) from the
                # PSUM via an accumulated -I @ hi matmul, evac the residual
                # on the (otherwise idle) Pool so the DVE keeps up with the
                # Schraudolph share of the exp stream.
                bias_ap = bq_sb[:, p : p + 1]
                if eng is None:
                    # pair 0 skips the lo residual entirely (zero-padded
                    # subtile): the hi->negid->lo ping-pong would gate the
                    # first exp tile. Only heads 0/1 carry the extra q
                    # rounding (~1.2e-2 local, still inside the gate).
                    nc.gpsimd.memset(qt[:, 1, :], 0.0)
                for t in range(NT):
                    ts_ = slice(t * 512, (t + 1) * 512)
                    ps = qp.tile([P, 512], f32, tag="qp")
                    for k in range(KC):
                        nc.tensor.matmul(
                            ps[:, :],
                            wq_sb[:, p, k, :],
                            hn[:, k, ts_],
                            start=(k == 0), stop=(k == KC - 1),
                        )
                    evac(eng, qt[:, 0, ts_], ps[:], bias_ap)
                    if eng is not None:
                        nc.tensor.matmul(
                            ps[:, :], negid8[:], qt[:, 0, ts_],
                            start=False, stop=True, skip_group_check=True,
                        )
                        evac(eng, qt[:, 1, ts_], ps[:], bias_ap)

            def conv_k(p, kt, eng, trange=range(NT)):
                bias_ap = bk_sb[:, p : p + 1]
                for t in trange:
                    ts_ = slice(t * 512, (t + 1) * 512)
                    ps = qp.tile([P, 512], f32, tag="qp")
                    for k in range(KC):
                        nc.tensor.matmul(
                            ps[:, :],
                            wk_sb[:, p, k, :],
                            hn[:, k, ts_],
                            start=(k == 0), stop=(k == KC - 1),
                        )
                    evac(eng, kt[:, 0, ts_], ps[:], bias_ap)
                    # duplicate into subtile 1 for the DoubleRow layout
                    nc.gpsimd.tensor_copy(out=kt[:, 1, ts_], in_=kt[:, 0, ts_])

            def v_tiles(half):
                # v^T tiles [s-part, head-major channel]; bv is folded into
                # the proj bias on the host, so no bias row here. Built in
                # halves slotted into heads 0 and 1 so the DVE evacuation
                # copies don't pile up in one stream window.
                for nt in range(4 * half, 4 * half + 4):
                    ps = qp.tile([P, 512], f32, tag="qp")
                    for k in range(KC):
                        nc.tensor.matmul(
                            ps[:, :],
                            hn[:, k, nt * P : (nt + 1) * P],
                            wv_sb[:, k, :],
                            start=(k == 0), stop=(k == KC - 1),
                        )
                    nc.vector.tensor_copy(
                        out=vt[:, nt, :, 0:CH],
                        in_=ps[:, :].rearrange("p (h c) -> p h c", h=NHEAD),
                    )

            # s-chunks routed to the DVE via the Schraudolph 2^x bit trick:
            # i16 = round(S*1024*log2(e) + (15*1024 - 62.2)) reinterpreted as
            # fp16 approximates exp(S) to ~+-4% -- the softmax denominator
            # uses the same approximated values, so the common mode cancels.
            SCHRA = (2, 5)
            SC1 = 1024 * 1.4426950408889634
            SC2 = 15360.0 - 62.2

            def head_st(h, qt, kt, mid=None):
                # S^T as fp8 DoubleRow: subtiles = (q_hi, q_lo) against a
                # duplicated k8, so each [128, 512] output costs 256 PE
                # cycles at near-fp16 accuracy.
                p, e = h // 2, h % 2
                rows = slice(64 * e, 64 * e + 64)
                exps = expp.tile([P, SC, N], f16, tag="exps")
                for sc in range(SC):
                    if sc == 2 and mid is not None:
                        mid()
                    if sc in SCHRA:
                        # Schraudolph tiles go through qp halves so the
                        # ACT-fed stp pipeline never waits on the DVE
                        for t in range(NT):
                            pq = qp.tile([P, 512], f32, tag="qp")
                            nc.tensor.matmul(
                                pq[:, :],
                                kt[rows, :, sc * P : (sc + 1) * P],
                                qt[rows, :, t * 512 : (t + 1) * 512],
                                start=True, stop=True,
                                perf_mode=PM.DoubleRow,
                            )
                            nc.vector.tensor_scalar(
                                exps[:, sc, t * 512 : (t + 1) * 512].bitcast(i16),
                                pq[:], SC1, SC2, OP.mult, OP.add,
                            )
                        continue
                    ps_st = stp.tile([P, N], f32, tag="stp")
                    for t in range(NT):
                        nc.tensor.matmul(
                            ps_st[:, t * 512 : (t + 1) * 512],
                            kt[rows, :, sc * P : (sc + 1) * P],
                            qt[rows, :, t * 512 : (t + 1) * 512],
                            start=True, stop=True,
                            perf_mode=PM.DoubleRow,
                        )
                    nc.scalar.activation(
                        out=exps[:, sc, :], in_=ps_st[:], func=AF.Exp
                    )
                return exps

            def head_a(h, exps, auT):
                # Transposed A: out a^T[t-part, ch] per 128-wide t-chunk, the
                # softmax denominator lands in column 64 as a per-partition
                # scalar -> one reciprocal + a fused normalize-evacuate.
                # (NOTE: the s accumulation must NOT interleave j groups --
                # PSUM allows one pending accumulation group per bank.)
                e = h % 2
                for u in range(2):
                    ps_aT = apool.tile([P, 260], f32, tag="apool")
                    pv = ps_aT[:].rearrange("p (j c) -> p j c", c=65)
                    for j in range(4):
                        tch = 4 * u + j
                        for sc in range(SC):
                            nc.tensor.matmul(
                                pv[:, j, :],
                                exps[:, sc, tch * P : (tch + 1) * P],
                                vt[:, sc, h, :],
                                start=(sc == 0), stop=(sc == SC - 1),
                            )
                    rcol = tmpp.tile([P, 4], f32, tag="rcol")
                    nc.vector.reciprocal(rcol[:], pv[:, :, 64])
                    for j in range(4):
                        tch = 4 * u + j
                        nc.vector.tensor_scalar(
                            auT[:, tch, 64 * e : 64 * e + 64], pv[:, j, 0:64],
                            rcol[:, j : j + 1], None, OP.mult,
                        )

            # ---------- qkv, then attention ----------
            def conv_pair(p, eng):
                qt = qpool.tile([P, 2, N], f8, tag="qt")
                kt = kpool.tile([P, 2, N], f8, tag="kt")
                conv_k(p, kt, eng, trange=(0,))
                conv_q(p, qt, eng)
                conv_k(p, kt, eng, trange=(1,))
                return qt, kt

            # pair 0 evacuates on the still-idle ACT so the stream starts
            # as early as possible; later pairs use the DVE.
            qt, kt = conv_pair(0, None)
            e = {}
            e[0] = head_st(0, qt, kt, mid=lambda: v_tiles(0))
            # each later conv pair is emitted a full head EARLY so its DVE
            # evacuations drain a head-window before the S^T that needs them
            # (they otherwise collide with that window's Schraudolph +
            # normalize work and stall the exp stream)
            nqt, nkt = conv_pair(1, nc.vector)
            # v^T tiles build on the PE while head 0/1's exps stream, slotted
            # into the middles of both heads' S^T so exp starts on time
            e[1] = head_st(1, qt, kt, mid=lambda: v_tiles(1))
            auTs = {}

            def transposes(p):
                # chunks 0/1 ride the XBAR mid-stream; chunks 2/3 take PE
                # identity-transposes -- the XBAR's HWDGE serialization
                # (~5us for 8 tiles + ~1us completion sem) would gate the
                # proj pre-runs right at the stream tail
                if p < KC - 2:
                    # XBAR transpose a^T -> a_u chunk p (SBUF->SBUF, no
                    # PE/DVE time)
                    for tch in range(SC):
                        nc.sync.dma_start_transpose(
                            a_u[:, p, tch * P : (tch + 1) * P], auTs[p][:, tch, :]
                        )
                else:
                    # tail chunk: PE transposes (identity matmul) -> shortest
                    # path into proj's k=3 contraction; the PSUM->SBUF copies
                    # run on the post-stream-idle ACT so the DVE (busy with
                    # the last head's normalize) isn't the serializer
                    for tch in range(SC):
                        ps_t = apool.tile([P, 260], f32, tag="apool")
                        pt = ps_t[:].bitcast(f16)
                        nc.tensor.matmul(
                            pt[:, 0:P], auTs[p][:, tch, :], ident_sb[:],
                            is_transpose=True,
                        )
                        if p == KC - 1:
                            # post-stream: ACT is idle, keep the DVE free for
                            # the bn/gn chains
                            nc.scalar.activation(
                                out=a_u[:, p, tch * P : (tch + 1) * P],
                                in_=pt[:, 0:P], func=AF.Copy,
                            )
                        else:
                            nc.vector.tensor_copy(
                                out=a_u[:, p, tch * P : (tch + 1) * P],
                                in_=pt[:, 0:P],
                            )

            # software pipeline: S^T/exp of pair p streams while the A
            # matmuls of pair p-1 drain, so pair boundaries stay dense
            for p in range(1, KC):
                qt, kt = nqt, nkt
                h0 = 2 * (p - 1)
                e[2 * p] = head_st(2 * p, qt, kt)
                if p < KC - 1:
                    nqt, nkt = conv_pair(p + 1, nc.vector)
                auT = autp.tile([P, SC, P], f16, tag="auT")
                auTs[p - 1] = auT
                head_a(h0, e[h0], auTs[p - 1])
                e[2 * p + 1] = head_st(2 * p + 1, qt, kt)
                head_a(h0 + 1, e[h0 + 1], auTs[p - 1])
                transposes(p - 1)
            auT = autp.tile([P, SC, P], f16, tag="auT")
            auTs[KC - 1] = auT
            head_a(6, e[6], auTs[KC - 1])

            head_a(7, e[7], auTs[KC - 1])

            # proj m0's k<3 accumulation also pre-runs under the stream tail
            preruns = {}
            ps_pm0 = stp.tile([P, N], f32, tag="stp")
            preruns[0] = ps_pm0
            for t in range(NT):
                for k in range(KC - 1):
                    nc.tensor.matmul(
                        preruns[0][:, t * 512 : (t + 1) * 512],
                        wp_sb[:, k, 0:P],
                        a_u[:, k, t * 512 : (t + 1) * 512],
                        start=(k == 0), stop=False,
                        skip_group_check=True,
                    )

            transposes(KC - 1)

            # ---------- proj + GN2 + residual, pipelined per chunk ----------
            # DVE bn_stats reads the proj PSUM directly (one pass for mean
            # and var); the per-chunk group reduce + affine then run on the
            # post-stream-idle PE/ACT, and one ACT Identity applies
            # y = ps*A + B straight out of PSUM. The (host-folded) proj bias
            # enters the stats and the B term via gn_chunk's bias_fold.
            mvs2 = big.tile([P, KC, 2], f32, tag="mvs2")
            # m0 and m1 finish their pre-run accumulators and take their
            # stats back-to-back BEFORE either gn chain runs, so the second
            # bn pass isn't stuck behind the first gn's PE/ACT hops in the
            # in-order DVE queue
            pr_halves = {}
            for m in preruns:
                halves = [
                    preruns[m][:, t * 512 : (t + 1) * 512] for t in range(NT)
                ]
                pr_halves[m] = halves
                stats = tmpp.tile([P, 2, 6], f32, tag="bnstats")
                for t in range(NT):
                    nc.tensor.matmul(
                        halves[t],
                        wp_sb[:, KC - 1, m * P : (m + 1) * P],
                        a_u[:, KC - 1, t * 512 : (t + 1) * 512],
                        start=False, stop=True,
                        skip_group_check=True,
                    )
                    nc.vector.bn_stats(out=stats[:, t, :], in_=halves[t])
                nc.vector.bn_aggr(out=mvs2[:, m, :], in_=stats[:])
            for m in range(KC):
                # m2 goes through two qp half-banks so it never waits on an
                # earlier chunk's apply to free stp; m3 recycles the first
                # freed stp buffer.
                if m in preruns:
                    halves = pr_halves[m]
                elif m == 2:
                    halves = []
                    for _t in range(NT):
                        psh = qp.tile([P, 512], f32, tag="qp")
                        halves.append(psh[:])
                else:
                    psm = stp.tile([P, N], f32, tag="stp")
                    halves = [psm[:, t * 512 : (t + 1) * 512] for t in range(NT)]
                if m not in preruns:
                    stats = tmpp.tile([P, 2, 6], f32, tag="bnstats")
                    for t in range(NT):
                        for k in range(KC):
                            nc.tensor.matmul(
                                halves[t],
                                wp_sb[:, k, m * P : (m + 1) * P],
                                a_u[:, k, t * 512 : (t + 1) * 512],
                                start=(k == 0),
                                stop=(k == KC - 1),
                                skip_group_check=True,
                            )
                        # stats on the finished half while the other runs
                        nc.vector.bn_stats(out=stats[:, t, :], in_=halves[t])
                    nc.vector.bn_aggr(out=mvs2[:, m, :], in_=stats[:])
                gn_chunk(m, mvs2[:, m, :], g2s, g2b, ab2,
                         bias_fold=bp_sb[:, m : m + 1])
                for t in range(NT):
                    nc.scalar.activation(
                        out=projf[:, m, t * 512 : (t + 1) * 512], in_=halves[t],
                        func=AF.Identity,
                        scale=ab2[:, m, 0:1], bias=ab2[:, m, 1:2],
                    )
                for t in range(NT):
                    nc.gpsimd.dma_start(
                        out_d[:, m, t * 512 : (t + 1) * 512],
                        projf[:, m, t * 512 : (t + 1) * 512],
                        accum_op=OP.add,
                    )

    nc.compile()
    return nc


def _host_prep(x, gn1_scale, gn1_bias, w_qkv, b_qkv, w_proj, b_proj, gn2_scale, gn2_bias):
    """Build per-core input maps (numpy only)."""
    f = np.float32
    bf = np.float16
    x = np.asarray(x, f)
    w_qkv = np.asarray(w_qkv, f)
    b_qkv = np.asarray(b_qkv, f)
    w_proj = np.asarray(w_proj, f)
    b_proj = np.asarray(b_proj, f)
    gn1_scale = np.asarray(gn1_scale, f)
    gn1_bias = np.asarray(gn1_bias, f)
    gn2_scale = np.asarray(gn2_scale, f)
    gn2_bias = np.asarray(gn2_bias, f)

    def chunk_vec(v):  # [C] -> [P, KC]
        return np.ascontiguousarray(v.reshape(KC, P).T)

    def chunk_mat(wt, dt=f):  # [C, M] -> [P, KC, M]
        return np.ascontiguousarray(wt.reshape(KC, P, -1).transpose(1, 0, 2).astype(dt))

    idx = np.arange(NHEAD)[:, None] * (3 * CH) + np.arange(CH)[None, :]
    q_idx, k_idx, v_idx = idx.ravel(), (idx + CH).ravel(), (idx + 2 * CH).ravel()

    s1 = float(CH) ** -0.25
    def mtile(w):  # [P, KC, C] -> [P, M, KC, P]
        return np.ascontiguousarray(
            w.reshape(P, KC, KC, P).transpose(0, 2, 1, 3)
        )
    wq = mtile(chunk_mat(w_qkv[q_idx].T * s1, bf))
    wk = mtile(chunk_mat(w_qkv[k_idx].T * s1, bf))
    wv = chunk_mat(w_qkv[v_idx].T, bf)
    wp = chunk_mat(w_proj.T, bf)
    wqk = np.ascontiguousarray(np.stack([wq, wk], axis=2))

    WI = 2 * KC * C
    WT = WI + P + P // 2
    wvp = np.zeros((P, WT), bf)
    wvp[:, 0 : KC * C] = wv.reshape(P, -1)
    wvp[:, KC * C : WI] = wp.reshape(P, -1)
    wvp[:, WI : WI + P] = np.eye(P, dtype=bf)
    # fp8e4m3 -I bit-packed into f16 lanes: -1.0 is 0xB8
    ni = np.zeros((P, P), np.uint8)
    ni[np.arange(P), np.arange(P)] = 0xB8
    wvp[:, WI + P : WT] = ni.view(np.uint16).view(bf)

    bq = chunk_vec(b_qkv[q_idx] * s1)
    bk = chunk_vec(b_qkv[k_idx] * s1)
    # v-bias folded into the proj bias: proj(a + bv) = proj(a) + w_proj @ bv
    bp_eff = b_proj + w_proj @ b_qkv[v_idx]
    bp = chunk_vec(bp_eff)

    cidx = np.arange(C)
    sel8 = np.zeros((P, KC, GC), f)
    sel8[cidx % P, cidx // P, (cidx % P) // 16] = 1.0 / 16.0
    selt8 = np.zeros((GC, KC, P), f)
    selt8[(cidx % P) // 16, cidx // P, cidx % P] = 1.0
    sel32 = np.zeros((P, KC, GROUPS), f)
    sel32[cidx % P, cidx // P, cidx // 16] = 1.0 / 16.0
    selt32 = np.zeros((GROUPS, KC, P), f)
    selt32[cidx // 16, cidx // P, cidx % P] = 1.0

    pars = np.zeros((P, 448), f)
    pars[:, 0:4] = chunk_vec(gn1_scale)
    pars[:, 4:8] = chunk_vec(gn1_bias)
    pars[:, 8:12] = bq
    pars[:, 12:16] = bk
    pars[:, 16:20] = bp
    pars[:, 20:24] = chunk_vec(gn2_scale)
    pars[:, 24:28] = chunk_vec(gn2_bias)
    pars[:, 28:60] = sel8.reshape(P, -1)
    pars[:, 60:188] = sel32.reshape(P, -1)
    pars[:, 444] = 1.0
    parsg = np.zeros((GROUPS, 1544), f)
    parsg[:, 0:512] = selt32.reshape(GROUPS, -1)
    parsg[:, 1024] = EPS
    parsg[0:GC, 1028:1540] = selt8.reshape(GC, -1)

    shared = {
        "wqk": wqk, "wvp": wvp, "pars": pars, "parsg": parsg,
    }
    in_maps = []
    for b in range(B):
        xb = np.ascontiguousarray(
            x[b].reshape(C, N).reshape(KC, P, N).transpose(1, 0, 2)
        )
        in_maps.append({"x": xb, "x8": xb.astype(bf), **shared})
    return in_maps


def _assemble(results):
    out = np.empty((B, C, H, W), np.float32)
    for b in range(B):
        ob = np.asarray(results[b]["out"])  # [P, KC, N]
        out[b] = ob.transpose(1, 0, 2).reshape(C, N).reshape(C, H, W)
    return out


def get_nc():
    if "nc" not in _CACHE:
        _CACHE["nc"] = _build_nc()
    return _CACHE["nc"]


def kernel(x, gn1_scale, gn1_bias, w_qkv, b_qkv, w_proj, b_proj, gn2_scale, gn2_bias):
    from concourse.bass_utils import run_bass_kernel_spmd

    nc = get_nc()
    in_maps = _host_prep(
        x, gn1_scale, gn1_bias, w_qkv, b_qkv, w_proj, b_proj, gn2_scale, gn2_bias
    )
    res = run_bass_kernel_spmd(nc, in_maps, core_ids=list(range(B)))
    return _assemble(res.results)


# revision 75
# speedup vs baseline: 1.0363x; 1.0145x over previous
"""Attention2d Trainium2 Bass kernel.

Reference computation (per batch element b of 8, one NeuronCore each):
    hn  = GroupNorm32(x) * gn1_scale + gn1_bias
    qkv = w_qkv @ hn + b_qkv          (1x1 conv == matmul over channels)
    per head h (8 heads, ch=64): q,k,v from qkv (torch reshape convention:
        head h uses rows h*192+{0..64,64..128,128..192})
    wgt = softmax((q*s)^T (k*s)), s = ch**-0.25
    a   = v @ wgt^T
    out = GroupNorm32(w_proj @ a + b_proj) ... * gn2_scale + gn2_bias
    y   = x + out

Device strategy (data-parallel over batch, 1 core per batch element):
  - ACT is the critical engine: the 8 heads' exp(S^T) stream is 64 tiles of
    [128, 1024] (~1.04us each). Everything else is scheduled so that stream
    never starves: PE work is held well under the stream duration.
  - S^T runs as fp8e4 DoubleRow matmuls at 0.5 cycles/row. To keep the
    quantization error inside the rel-err budget, q ships as TWO fp8
    k-subtiles (hi + residual lo, together fp16-accurate) against a
    DUPLICATED fp8 k: S = k8^T(q_hi + q_lo), so only k's single fp8
    rounding touches the logits. q_lo is formed by accumulating -I @ q_hi
    into the conv PSUM (one cheap fp8 matmul) and re-evacuating.
  - exps, v^T tiles, the A matmuls, convs and proj all stay fp16: their
    quantization hits the output directly (measured: fp8 exps alone cost
    2.6e-2 rel err), while the k-side fp8 is dampened through softmax.
  - the A matmul runs TRANSPOSED: out a^T[t-part, ch] per 128-wide t-chunk;
    the softmax denominator (ones-column of v^T) lands in column 64 as a
    per-partition scalar: one DVE reciprocal + one fused tensor_scalar
    normalizes while evacuating. The LAST head's A accumulation is split
    into s-halves so only half of it trails the final exp tile.
  - a^T -> a via XBAR dma_start_transpose (no engine time); the LAST pair
    uses PE identity-transposes so proj isn't tail-gated by the HWDGE queue
  - v-bias folded into the proj bias on the host; proj bias folded into the
    GN2 affine + channel stats
  - GN2: DVE bn_stats reads each proj PSUM tile directly (one pass), the
    group reduce + affine run on PE/ACT (idle post-stream), and a single
    ACT Identity applies y = ps*A + B straight out of PSUM
  - input DMAs are packed and ordered so GN1 stats chase the x chunks
    (ACT takes the chunk that lands first, DVE the rest), pair-0 weights
    ship in their own small DMA, and pair-0's q/k evacuations run on the
    still-idle ACT so the first exp fires as early as possible
  - scratch warm-up matmuls on a memset tile hold the PE in its fast
    p-state from t~0
"""

import numpy as np

NHEAD = 8
GROUPS = 32
EPS = 1e-5
B, C, H, W = 8, 512, 32, 32
N = H * W            # 1024 spatial positions
CH = C // NHEAD      # 64 channels per head
P = 128              # partitions
KC = C // P          # 4 channel chunks
NT = N // 512        # 2 column tiles of 512
SC = N // P          # 8 s-chunks
GC = GROUPS // KC    # 8 groups per chunk

_CACHE = {}


def _build_nc():
    import concourse.tile as tile
    from concourse import mybir, bacc
    from concourse.hw_specs import get_activation_tables

    f32 = mybir.dt.float32
    f16 = mybir.dt.float16
    f8 = mybir.dt.float8e4
    i16 = mybir.dt.int16
    AF = mybir.ActivationFunctionType
    OP = mybir.AluOpType
    PM = mybir.MatmulPerfMode

    nc = bacc.Bacc("TRN2", target_bir_lowering=False, num_devices=8)

    WI = 2 * KC * C          # wv/wp block columns in the packed weight tile
    WT = WI + P + P // 2     # + f16 identity + fp8 -identity (bit-packed)

    x_d = nc.dram_tensor("x", [P, KC, N], f32, kind="ExternalInput")
    x8_d = nc.dram_tensor("x8", [P, KC, N], f16, kind="ExternalInput")
    wqk_d = nc.dram_tensor("wqk", [P, KC, 2, KC, P], f16, kind="ExternalInput")
    wvp_d = nc.dram_tensor("wvp", [P, WT], f16, kind="ExternalInput")
    pars_d = nc.dram_tensor("pars", [P, 448], f32, kind="ExternalInput")
    parsg_d = nc.dram_tensor("parsg", [GROUPS, 1544], f32, kind="ExternalInput")
    out_d = nc.dram_tensor("out", [P, KC, N], f32, kind="ExternalOutput")

    with tile.TileContext(nc) as tc:
        with (
            tc.tile_pool(name="big", bufs=1) as big,
            tc.tile_pool(name="wpool", bufs=2) as wpool,
            tc.tile_pool(name="qpool", bufs=2) as qpool,
            tc.tile_pool(name="kpool", bufs=4) as kpool,
            tc.tile_pool(name="vtp", bufs=1) as vtp,
            tc.tile_pool(name="expp", bufs=3) as expp,
            tc.tile_pool(name="autp", bufs=2) as autp,
            tc.tile_pool(name="tmpp", bufs=2) as tmpp,
            tc.tile_pool(name="stp", bufs=2, space="PSUM") as stp,
            tc.tile_pool(name="apool", bufs=2, space="PSUM") as apool,
            tc.tile_pool(name="qp", bufs=2, space="PSUM") as qp,
        ):
            # ---------- persistent SBUF tiles ----------
            x_sb = big.tile([P, KC, N], f16, tag="x_sb")
            hn = big.tile([P, KC, N], f16, tag="hn")
            vt = vtp.tile([P, SC, NHEAD, CH + 1], f16, tag="vp")
            pars_sb = big.tile([P, 448], f32, tag="pars_sb")
            parsg_sb = big.tile([GROUPS, 1544], f32, tag="parsg_sb")
            ab1 = big.tile([P, KC, 2], f32, tag="ab1")
            ab2 = big.tile([P, KC, 2], f32, tag="ab2")
            projf = big.tile([P, KC, N], f32, tag="projf")
            a_u = big.tile([P, KC, N], f16, tag="a_u")
            scr16 = big.tile([P, 2, N], f16, tag="scr16")
            wmup = big.tile([P, 512], f16, tag="wmup")

            # packed-parameter views
            g1s = pars_sb[:, 0:4]
            g1b = pars_sb[:, 4:8]
            bq_sb = pars_sb[:, 8:12]
            bk_sb = pars_sb[:, 12:16]
            bp_sb = pars_sb[:, 16:20]
            g2s = pars_sb[:, 20:24]
            g2b = pars_sb[:, 24:28]
            sel8 = pars_sb[:, 28:60].rearrange("p (k g) -> p k g", k=KC)
            sel32 = pars_sb[:, 60:188].rearrange("p (k g) -> p k g", k=KC)
            onescol = pars_sb[:, 444:445]
            selt32 = parsg_sb[:, 0:512].rearrange("g (k c) -> g k c", k=KC)
            selt8 = parsg_sb[0:GC, 1028:1540].rearrange("g (k c) -> g k c", k=KC)
            eps32 = parsg_sb[:, 1024:1025]
            eps8 = parsg_sb[0:GC, 1024:1025]

            # PE warm-up from t~0: matmuls on a Pool-memset scratch tile hold
            # the PE through its p-state ramp so the first real matmuls run at
            # full speed. Results are discarded.
            nc.gpsimd.memset(wmup[:], 0.0)
            # enough back-to-back warm-up matmuls to bridge to the first GN1
            # group matmuls (~8us) -- a >~2us PE idle gap resets the ramp
            for _ in range(26):
                ps_w = qp.tile([P, 512], f32, tag="qp")
                nc.tensor.matmul(
                    ps_w[:, 0:448],
                    wmup[:, 0:128],
                    wmup[:, 0:448],
                    start=True, stop=True,
                )

            # ---------- input DMAs, ordered for the GN1 -> conv chain -------
            # chunk 2 lands first (its stats run on ACT), then chunk 3 so the
            # DVE's last bn_stats isn't the straggler; pair-0 weights ship in
            # their own small contiguous DMA.
            nc.sync.dma_start(x_sb[:, 2, :], x8_d[:, 2, :])
            nc.sync.dma_start(x_sb[:, 3, :], x8_d[:, 3, :])
            nc.sync.dma_start(x_sb[:, 0, :], x8_d[:, 0, :])
            nc.sync.dma_start(x_sb[:, 1, :], x8_d[:, 1, :])
            nc.sync.dma_start(pars_sb[:], pars_d[:])
            nc.sync.dma_start(parsg_sb[:], parsg_d[:])
            wqk_sb = wpool.tile([P, KC, 2, KC, P], f16, tag="wqk")
            nc.sync.dma_start(wqk_sb[:, 0], wqk_d[:, 0])
            nc.sync.dma_start(wqk_sb[:, 1:KC], wqk_d[:, 1:KC])
            wvp_sb = wpool.tile([P, WT], f16, tag="wvp")
            nc.sync.dma_start(wvp_sb[:], wvp_d[:])
            nc.sync.dma_start(out_d[:], x_d[:])

            wq_sb = wqk_sb[:, :, 0]
            wk_sb = wqk_sb[:, :, 1]
            wv_sb = wvp_sb[:, 0 : KC * C].rearrange("p (k c) -> p k c", k=KC)
            wp_sb = wvp_sb[:, KC * C : WI].rearrange("p (k c) -> p k c", k=KC)
            ident_sb = wvp_sb[:, WI : WI + P]
            negid8 = wvp_sb[:, WI + P : WT].bitcast(f8)

            # Preload the combined ln+exp ACT table set once (Ln/Exp are used
            # for the GroupNorm rstd), so the bacc table-load pass doesn't
            # thrash between sets.
            _set_names = list(get_activation_tables(nc.m.arch).keys())
            _tl = mybir.InstLoadActFuncSet(
                name=nc.get_next_instruction_name(),
                ins=[],
                outs=[],
                act_func_set_id=_set_names.index("natural_log_exp_and_others"),
            )
            _tl.engine = mybir.EngineType.Activation
            nc.scalar.add_instruction(_tl)

            nc.gpsimd.tensor_copy(
                out=vt[:, :, :, CH : CH + 1],
                in_=onescol[:, :, None, None].to_broadcast((P, SC, NHEAD, 1)),
            )

            # ---------- per-chunk GroupNorm chain (used by GN2) ----------
            def gn_chunk(k, mvs_ap, gs, gb, ab, bias_fold):
                """ab[:, k] = per-channel (A, B) for y = src*A + B, given
                per-channel (mean, var) in mvs_ap ([P, 2], chunk k); the
                [P, 1] AP bias_fold adjusts the stats and B as if it had been
                added to the source. DVE ops read the group-reduce PSUM
                directly to keep the dependency chain short."""
                stat2 = tmpp.tile([P, 2], f32, tag="stat2")
                nc.vector.tensor_tensor(
                    stat2[:, 0:1], mvs_ap[:, 0:1], bias_fold, OP.add
                )
                musq = tmpp.tile([P, 1], f32, tag="musq")
                nc.vector.tensor_tensor(musq[:], stat2[:, 0:1], stat2[:, 0:1], OP.mult)
                nc.vector.tensor_tensor(stat2[:, 1:2], mvs_ap[:, 1:2], musq[:], OP.add)
                ps_g = apool.tile([P, 260], f32, tag="apool")
                nc.tensor.matmul(
                    ps_g[0:GC, 0:2], sel8[:, k, :], stat2[:, :],
                    start=True, stop=True,
                )
                gstat = tmpp.tile([GC, 2], f32, tag="gstat")
                nc.vector.tensor_copy(out=gstat[:, 0:1], in_=ps_g[0:GC, 0:1])
                gvar = tmpp.tile([GC, 1], f32, tag="gvar")
                gmusq = tmpp.tile([GC, 1], f32, tag="gmusq")
                # only one PSUM operand allowed per instruction: square the
                # SBUF copy of the group mean
                nc.vector.tensor_tensor(gmusq[:], gstat[:, 0:1], gstat[:, 0:1], OP.mult)
                nc.vector.tensor_tensor(gvar[:], ps_g[0:GC, 1:2], gmusq[:], OP.subtract)
                nc.scalar.activation(out=gvar[:], in_=gvar[:], func=AF.Ln, bias=eps8)
                nc.scalar.activation(out=gstat[:, 1:2], in_=gvar[:], func=AF.Exp, scale=-0.5)
                ps_c = apool.tile([P, 260], f32, tag="apool")
                nc.tensor.matmul(
                    ps_c[:, 0:2], selt8[:, k, :], gstat[:],
                    start=True, stop=True,
                )
                nc.vector.tensor_tensor(
                    ab[:, k, 0:1], gs[:, k : k + 1], ps_c[:, 1:2], OP.mult
                )
                # B = gb + A*(bias_fold - mean_c)
                ma = tmpp.tile([P, 2], f32, tag="ma")
                nc.vector.tensor_tensor(ma[:, 0:1], bias_fold, ps_c[:, 0:1], OP.subtract)
                nc.vector.tensor_tensor(ma[:, 1:2], ab[:, k, 0:1], ma[:, 0:1], OP.mult)
                nc.vector.tensor_tensor(
                    ab[:, k, 1:2], gb[:, k : k + 1], ma[:, 1:2], OP.add
                )

            # ---------- GN1 -> hn (stats chase the x chunk DMAs: ACT
            # accumulators for chunk 2 (lands first), DVE bn_stats for chunks
            # 3,0,1 in arrival order; one batched group reduce + affine) -----
            BN_CHUNKS = (3, 0, 1)
            mvs1 = big.tile([P, 3, 2], f32, tag="mvs1")
            stat2 = big.tile([P, KC, 2], f32, tag="stat2b")
            nc.scalar.activation(
                out=scr16[:, 0, :], in_=x_sb[:, 2, :], func=AF.Copy,
                scale=1.0 / N, accum_out=stat2[:, 2, 0:1],
            )
            nc.scalar.activation(
                out=scr16[:, 1, :], in_=x_sb[:, 2, :], func=AF.Square,
                scale=1.0 / 32.0, accum_out=stat2[:, 2, 1:2],
            )
            for i, k in enumerate(BN_CHUNKS):
                stats = tmpp.tile([P, 2, 6], f32, tag="bnstats")
                resh = x_sb[:, k, :].rearrange("p (s f) -> p s f", f=512)
                for si in range(2):
                    nc.vector.bn_stats(out=stats[:, si, :], in_=resh[:, si, :])
                nc.vector.bn_aggr(out=mvs1[:, i, :], in_=stats[:])
            musq = tmpp.tile([P, 3], f32, tag="musqb")
            nc.vector.tensor_tensor(musq[:], mvs1[:, :, 0], mvs1[:, :, 0], OP.mult)
            for i, k in enumerate(BN_CHUNKS):
                nc.vector.tensor_tensor(
                    stat2[:, k, 1:2], mvs1[:, i, 1:2], musq[:, i : i + 1], OP.add
                )
                nc.vector.tensor_copy(out=stat2[:, k, 0:1], in_=mvs1[:, i, 0:1])
            ps_g1 = qp.tile([P, 512], f32, tag="qp")
            for k in range(KC):
                nc.tensor.matmul(
                    ps_g1[0:GROUPS, 0:2], sel32[:, k, :], stat2[:, k, :],
                    start=(k == 0), stop=(k == KC - 1),
                )
            gst1 = big.tile([GROUPS, 2], f32, tag="gst1")
            gms1 = tmpp.tile([GROUPS, 2], f32, tag="gms1")
            nc.vector.tensor_copy(out=gms1[:], in_=ps_g1[0:GROUPS, 0:2])
            nc.vector.tensor_copy(out=gst1[:, 0:1], in_=gms1[:, 0:1])
            gv1 = tmpp.tile([GROUPS, 1], f32, tag="gv1")
            gmu1 = tmpp.tile([GROUPS, 1], f32, tag="gmu1")
            nc.vector.tensor_tensor(gmu1[:], gms1[:, 0:1], gms1[:, 0:1], OP.mult)
            nc.vector.tensor_tensor(gv1[:], gms1[:, 1:2], gmu1[:], OP.subtract)
            nc.scalar.activation(out=gv1[:], in_=gv1[:], func=AF.Ln, bias=eps32)
            nc.scalar.activation(out=gst1[:, 1:2], in_=gv1[:], func=AF.Exp, scale=-0.5)
            ps_c1 = qp.tile([P, 512], f32, tag="qp")
            for k in range(KC):
                nc.tensor.matmul(
                    ps_c1[:, 2 * k : 2 * k + 2], selt32[:, k, :], gst1[:],
                    start=True, stop=True,
                )
            cst1 = tmpp.tile([P, KC, 2], f32, tag="cst1")
            nc.vector.tensor_copy(out=cst1[:], in_=ps_c1[:, 0 : 2 * KC])
            nc.vector.tensor_tensor(ab1[:, :, 0], g1s[:, :], cst1[:, :, 1], OP.mult)
            ma1 = tmpp.tile([P, KC], f32, tag="ma1")
            nc.vector.tensor_tensor(ma1[:], cst1[:, :, 0], ab1[:, :, 0], OP.mult)
            nc.vector.tensor_tensor(ab1[:, :, 1], g1b[:, :], ma1[:], OP.subtract)
            for k in range(KC):
                nc.vector.tensor_scalar(
                    hn[:, k, :], x_sb[:, k, :],
                    ab1[:, k, 0:1], ab1[:, k, 1:2], OP.mult, OP.add,
                )

            # ---------- phase helpers ----------
            def evac(eng, dst, src, bias_ap):
                if eng is None:
                    # ACT bias-add copy (idle pre-stream)
                    nc.scalar.activation(
                        out=dst, in_=src, func=AF.Identity, bias=bias_ap
                    )
                else:
                    eng.tensor_scalar(dst, src, bias_ap, None, OP.add)

            def conv_q(p, qt, eng):
                # q as hi+lo fp8 pair: evac hi, subtract rne8(hi) from the
                # PSUM via an accumulated -I @ hi matmul, evac the residual
                # on the (otherwise idle) Pool so the DVE keeps up with the
                # Schraudolph share of the exp stream.
                bias_ap = bq_sb[:, p : p + 1]
                if eng is None:
                    # pair 0 skips the lo residual entirely (zero-padded
                    # subtile): the hi->negid->lo ping-pong would gate the
                    # first exp tile. Only heads 0/1 carry the extra q
                    # rounding (~1.2e-2 local, still inside the gate).
                    nc.gpsimd.memset(qt[:, 1, :], 0.0)
                for t in range(NT):
                    ts_ = slice(t * 512, (t + 1) * 512)
                    ps = qp.tile([P, 512], f32, tag="qp")
                    for k in range(KC):
                        nc.tensor.matmul(
                            ps[:, :],
                            wq_sb[:, p, k, :],
                            hn[:, k, ts_],
                            start=(k == 0), stop=(k == KC - 1),
                        )
                    evac(eng, qt[:, 0, ts_], ps[:], bias_ap)
                    if eng is not None:
                        nc.tensor.matmul(
                            ps[:, :], negid8[:], qt[:, 0, ts_],
                            start=False, stop=True, skip_group_check=True,
                        )
                        evac(eng, qt[:, 1, ts_], ps[:], bias_ap)

            def conv_k(p, kt, eng, trange=range(NT)):
                bias_ap = bk_sb[:, p : p + 1]
                for t in trange:
                    ts_ = slice(t * 512, (t + 1) * 512)
                    ps = qp.tile([P, 512], f32, tag="qp")
                    for k in range(KC):
                        nc.tensor.matmul(
                            ps[:, :],
                            wk_sb[:, p, k, :],
                            hn[:, k, ts_],
                            start=(k == 0), stop=(k == KC - 1),
                        )
                    evac(eng, kt[:, 0, ts_], ps[:], bias_ap)
                    # duplicate into subtile 1 for the DoubleRow layout
                    nc.gpsimd.tensor_copy(out=kt[:, 1, ts_], in_=kt[:, 0, ts_])

            def v_tiles(half):
                # v^T tiles [s-part, head-major channel]; bv is folded into
                # the proj bias on the host, so no bias row here. Built in
                # halves slotted into heads 0 and 1 so the DVE evacuation
                # copies don't pile up in one stream window.
                for nt in range(4 * half, 4 * half + 4):
                    ps = qp.tile([P, 512], f32, tag="qp")
                    for k in range(KC):
                        nc.tensor.matmul(
                            ps[:, :],
                            hn[:, k, nt * P : (nt + 1) * P],
                            wv_sb[:, k, :],
                            start=(k == 0), stop=(k == KC - 1),
                        )
                    nc.vector.tensor_copy(
                        out=vt[:, nt, :, 0:CH],
                        in_=ps[:, :].rearrange("p (h c) -> p h c", h=NHEAD),
                    )

            # s-chunks routed to the DVE via the Schraudolph 2^x bit trick:
            # i16 = round(S*1024*log2(e) + (15*1024 - 62.2)) reinterpreted as
            # fp16 approximates exp(S) to ~+-4% -- the softmax denominator
            # uses the same approximated values, so the common mode cancels.
            SCHRA = (2, 5)
            SC1 = 1024 * 1.4426950408889634
            SC2 = 15360.0 - 62.2

            def head_st(h, qt, kt, mid=None):
                # S^T as fp8 DoubleRow: subtiles = (q_hi, q_lo) against a
                # duplicated k8, so each [128, 512] output costs 256 PE
                # cycles at near-fp16 accuracy.
                p, e = h // 2, h % 2
                rows = slice(64 * e, 64 * e + 64)
                exps = expp.tile([P, SC, N], f16, tag="exps")
                for sc in range(SC):
                    if sc == 2 and mid is not None:
                        mid()
                    if sc in SCHRA:
                        # Schraudolph tiles go through qp halves so the
                        # ACT-fed stp pipeline never waits on the DVE
                        for t in range(NT):
                            pq = qp.tile([P, 512], f32, tag="qp")
                            nc.tensor.matmul(
                                pq[:, :],
                                kt[rows, :, sc * P : (sc + 1) * P],
                                qt[rows, :, t * 512 : (t + 1) * 512],
                                start=True, stop=True,
                                perf_mode=PM.DoubleRow,
                            )
                            nc.vector.tensor_scalar(
                                exps[:, sc, t * 512 : (t + 1) * 512].bitcast(i16),
                                pq[:], SC1, SC2, OP.mult, OP.add,
                            )
                        continue
                    ps_st = stp.tile([P, N], f32, tag="stp")
                    for t in range(NT):
                        nc.tensor.matmul(
                            ps_st[:, t * 512 : (t + 1) * 512],
                            kt[rows, :, sc * P : (sc + 1) * P],
                            qt[rows, :, t * 512 : (t + 1) * 512],
                            start=True, stop=True,
                            perf_mode=PM.DoubleRow,
                        )
                    nc.scalar.activation(
                        out=exps[:, sc, :], in_=ps_st[:], func=AF.Exp
                    )
                return exps

            def head_a(h, exps, auT):
                # Transposed A: out a^T[t-part, ch] per 128-wide t-chunk, the
                # softmax denominator lands in column 64 as a per-partition
                # scalar -> one reciprocal + a fused normalize-evacuate.
                # (NOTE: the s accumulation must NOT interleave j groups --
                # PSUM allows one pending accumulation group per bank.)
                e = h % 2
                for u in range(2):
                    ps_aT = apool.tile([P, 260], f32, tag="apool")
                    pv = ps_aT[:].rearrange("p (j c) -> p j c", c=65)
                    for j in range(4):
                        tch = 4 * u + j
                        for sc in range(SC):
                            nc.tensor.matmul(
                                pv[:, j, :],
                                exps[:, sc, tch * P : (tch + 1) * P],
                                vt[:, sc, h, :],
                                start=(sc == 0), stop=(sc == SC - 1),
                            )
                    rcol = tmpp.tile([P, 4], f32, tag="rcol")
                    nc.vector.reciprocal(rcol[:], pv[:, :, 64])
                    for j in range(4):
                        tch = 4 * u + j
                        nc.vector.tensor_scalar(
                            auT[:, tch, 64 * e : 64 * e + 64], pv[:, j, 0:64],
                            rcol[:, j : j + 1], None, OP.mult,
                        )

            # ---------- qkv, then attention ----------
            def conv_pair(p, eng):
                qt = qpool.tile([P, 2, N], f8, tag="qt")
                kt = kpool.tile([P, 2, N], f8, tag="kt")
                conv_k(p, kt, eng, trange=(0,))
                conv_q(p, qt, eng)
                conv_k(p, kt, eng, trange=(1,))
                return qt, kt

            # pair 0 evacuates on the still-idle ACT so the stream starts
            # as early as possible; later pairs use the DVE.
            qt, kt = conv_pair(0, None)
            e = {}
            e[0] = head_st(0, qt, kt, mid=lambda: v_tiles(0))
            # each later conv pair is emitted a full head EARLY so its DVE
            # evacuations drain a head-window before the S^T that needs them
            # (they otherwise collide with that window's Schraudolph +
            # normalize work and stall the exp stream)
            nqt, nkt = conv_pair(1, nc.vector)
            # v^T tiles build on the PE while head 0/1's exps stream, slotted
            # into the middles of both heads' S^T so exp starts on time
            e[1] = head_st(1, qt, kt, mid=lambda: v_tiles(1))
            auTs = {}

            def transposes(p):
                # chunks 0/1 ride the XBAR mid-stream; chunks 2/3 take PE
                # identity-transposes -- the XBAR's HWDGE serialization
                # (~5us for 8 tiles + ~1us completion sem) would gate the
                # proj pre-runs right at the stream tail
                if p < KC - 2:
                    # XBAR transpose a^T -> a_u chunk p (SBUF->SBUF, no
                    # PE/DVE time)
                    for tch in range(SC):
                        nc.sync.dma_start_transpose(
                            a_u[:, p, tch * P : (tch + 1) * P], auTs[p][:, tch, :]
                        )
                else:
                    # tail chunk: PE transposes (identity matmul) -> shortest
                    # path into proj's k=3 contraction; the PSUM->SBUF copies
                    # run on the post-stream-idle ACT so the DVE (busy with
                    # the last head's normalize) isn't the serializer
                    for tch in range(SC):
                        ps_t = apool.tile([P, 260], f32, tag="apool")
                        pt = ps_t[:].bitcast(f16)
                        nc.tensor.matmul(
                            pt[:, 0:P], auTs[p][:, tch, :], ident_sb[:],
                            is_transpose=True,
                        )
                        if p == KC - 1:
                            # post-stream: ACT is idle, keep the DVE free for
                            # the bn/gn chains
                            nc.scalar.activation(
                                out=a_u[:, p, tch * P : (tch + 1) * P],
                                in_=pt[:, 0:P], func=AF.Copy,
                            )
                        else:
                            nc.vector.tensor_copy(
                                out=a_u[:, p, tch * P : (tch + 1) * P],
                                in_=pt[:, 0:P],
                            )

            # software pipeline: S^T/exp of pair p streams while the A
            # matmuls of pair p-1 drain, so pair boundaries stay dense
            for p in range(1, KC):
                qt, kt = nqt, nkt
                h0 = 2 * (p - 1)
                e[2 * p] = head_st(2 * p, qt, kt)
                if p < KC - 1:
                    nqt, nkt = conv_pair(p + 1, nc.vector)
                auT = autp.tile([P, SC, P], f16, tag="auT")
                auTs[p - 1] = auT
                head_a(h0, e[h0], auTs[p - 1])
                e[2 * p + 1] = head_st(2 * p + 1, qt, kt)
                head_a(h0 + 1, e[h0 + 1], auTs[p - 1])
                transposes(p - 1)
            auT = autp.tile([P, SC, P], f16, tag="auT")
            auTs[KC - 1] = auT
            head_a(6, e[6], auTs[KC - 1])

            # proj m0's k<3 accumulation pre-runs under the stream tail
            preruns = {}
            ps_pm0 = stp.tile([P, N], f32, tag="stp")
            preruns[0] = ps_pm0
            for t in range(NT):
                for k in range(KC - 1):
                    nc.tensor.matmul(
                        preruns[0][:, t * 512 : (t + 1) * 512],
                        wp_sb[:, k, 0:P],
                        a_u[:, k, t * 512 : (t + 1) * 512],
                        start=(k == 0), stop=False,
                        skip_group_check=True,
                    )

            # head 7's A u-blocks (apool) interleave with transpose halves
            # (qp banks -- free here -- so they don't contend with the A
            # accumulators): m0's k3-t0 only needs a_u[3] tch 0-3, which
            # lands right after u0's normalize
            for u in range(2):
                ps_a7 = apool.tile([P, 260], f32, tag="apool")
                pv7 = ps_a7[:].rearrange("p (j c) -> p j c", c=65)
                for j in range(4):
                    tch = 4 * u + j
                    for sc in range(SC):
                        nc.tensor.matmul(
                            pv7[:, j, :],
                            e[7][:, sc, tch * P : (tch + 1) * P],
                            vt[:, sc, 7, :],
                            start=(sc == 0), stop=(sc == SC - 1),
                        )
                rcol7 = tmpp.tile([P, 4], f32, tag="rcol")
                nc.vector.reciprocal(rcol7[:], pv7[:, :, 64])
                for j in range(4):
                    tch = 4 * u + j
                    nc.vector.tensor_scalar(
                        auTs[KC - 1][:, tch, 64:128], pv7[:, j, 0:64],
                        rcol7[:, j : j + 1], None, OP.mult,
                    )
                for tch in range(4 * u, 4 * u + 4):
                    ps_t7 = qp.tile([P, 512], f32, tag="qp")
                    pt7 = ps_t7[:, 0:260].bitcast(f16)
                    nc.tensor.matmul(
                        pt7[:, 0:P], auTs[KC - 1][:, tch, :], ident_sb[:],
                        is_transpose=True,
                    )
                    nc.scalar.activation(
                        out=a_u[:, KC - 1, tch * P : (tch + 1) * P],
                        in_=pt7[:, 0:P], func=AF.Copy,
                    )

            # ---------- proj + GN2 + residual, pipelined per chunk ----------
            # DVE bn_stats reads the proj PSUM directly (one pass for mean
            # and var); the per-chunk group reduce + affine then run on the
            # post-stream-idle PE/ACT, and one ACT Identity applies
            # y = ps*A + B straight out of PSUM. The (host-folded) proj bias
            # enters the stats and the B term via gn_chunk's bias_fold.
            mvs2 = big.tile([P, KC, 2], f32, tag="mvs2")
            # m0 and m1 finish their pre-run accumulators and take their
            # stats back-to-back BEFORE either gn chain runs, so the second
            # bn pass isn't stuck behind the first gn's PE/ACT hops in the
            # in-order DVE queue
            pr_halves = {}
            for m in preruns:
                halves = [
                    preruns[m][:, t * 512 : (t + 1) * 512] for t in range(NT)
                ]
                pr_halves[m] = halves
                stats = tmpp.tile([P, 2, 6], f32, tag="bnstats")
                for t in range(NT):
                    nc.tensor.matmul(
                        halves[t],
                        wp_sb[:, KC - 1, m * P : (m + 1) * P],
                        a_u[:, KC - 1, t * 512 : (t + 1) * 512],
                        start=False, stop=True,
                        skip_group_check=True,
                    )
                    nc.vector.bn_stats(out=stats[:, t, :], in_=halves[t])
                nc.vector.bn_aggr(out=mvs2[:, m, :], in_=stats[:])
            for m in range(KC):
                # m2 goes through two qp half-banks so it never waits on an
                # earlier chunk's apply to free stp; m3 recycles the first
                # freed stp buffer.
                if m in preruns:
                    halves = pr_halves[m]
                elif m == 2:
                    halves = []
                    for _t in range(NT):
                        psh = qp.tile([P, 512], f32, tag="qp")
                        halves.append(psh[:])
                else:
                    psm = stp.tile([P, N], f32, tag="stp")
                    halves = [psm[:, t * 512 : (t + 1) * 512] for t in range(NT)]
                if m not in preruns:
                    stats = tmpp.tile([P, 2, 6], f32, tag="bnstats")
                    for t in range(NT):
                        for k in range(KC):
                            nc.tensor.matmul(
                                halves[t],
                                wp_sb[:, k, m * P : (m + 1) * P],
                                a_u[:, k, t * 512 : (t + 1) * 512],
                                start=(k == 0),
                                stop=(k == KC - 1),
                                skip_group_check=True,
                            )
                        # stats on the finished half while the other runs
                        nc.vector.bn_stats(out=stats[:, t, :], in_=halves[t])
                    nc.vector.bn_aggr(out=mvs2[:, m, :], in_=stats[:])
                gn_chunk(m, mvs2[:, m, :], g2s, g2b, ab2,
                         bias_fold=bp_sb[:, m : m + 1])
                for t in range(NT):
                    nc.scalar.activation(
                        out=projf[:, m, t * 512 : (t + 1) * 512], in_=halves[t],
                        func=AF.Identity,
                        scale=ab2[:, m, 0:1], bias=ab2[:, m, 1:2],
                    )
                for t in range(NT):
                    nc.gpsimd.dma_start(
                        out_d[:, m, t * 512 : (t + 1) * 512],
                        projf[:, m, t * 512 : (t + 1) * 512],
                        accum_op=OP.add,
                    )

    nc.compile()
    return nc


def _host_prep(x, gn1_scale, gn1_bias, w_qkv, b_qkv, w_proj, b_proj, gn2_scale, gn2_bias):
    """Build per-core input maps (numpy only)."""
    f = np.float32
    bf = np.float16
    x = np.asarray(x, f)
    w_qkv = np.asarray(w_qkv, f)
    b_qkv = np.asarray(b_qkv, f)
    w_proj = np.asarray(w_proj, f)
    b_proj = np.asarray(b_proj, f)
    gn1_scale = np.asarray(gn1_scale, f)
    gn1_bias = np.asarray(gn1_bias, f)
    gn2_scale = np.asarray(gn2_scale, f)
    gn2_bias = np.asarray(gn2_bias, f)

    def chunk_vec(v):  # [C] -> [P, KC]
        return np.ascontiguousarray(v.reshape(KC, P).T)

    def chunk_mat(wt, dt=f):  # [C, M] -> [P, KC, M]
        return np.ascontiguousarray(wt.reshape(KC, P, -1).transpose(1, 0, 2).astype(dt))

    idx = np.arange(NHEAD)[:, None] * (3 * CH) + np.arange(CH)[None, :]
    q_idx, k_idx, v_idx = idx.ravel(), (idx + CH).ravel(), (idx + 2 * CH).ravel()

    s1 = float(CH) ** -0.25
    def mtile(w):  # [P, KC, C] -> [P, M, KC, P]
        return np.ascontiguousarray(
            w.reshape(P, KC, KC, P).transpose(0, 2, 1, 3)
        )
    wq = mtile(chunk_mat(w_qkv[q_idx].T * s1, bf))
    wk = mtile(chunk_mat(w_qkv[k_idx].T * s1, bf))
    wv = chunk_mat(w_qkv[v_idx].T, bf)
    wp = chunk_mat(w_proj.T, bf)
    wqk = np.ascontiguousarray(np.stack([wq, wk], axis=2))

    WI = 2 * KC * C
    WT = WI + P + P // 2
    wvp = np.zeros((P, WT), bf)
    wvp[:, 0 : KC * C] = wv.reshape(P, -1)
    wvp[:, KC * C : WI] = wp.reshape(P, -1)
    wvp[:, WI : WI + P] = np.eye(P, dtype=bf)
    # fp8e4m3 -I bit-packed into f16 lanes: -1.0 is 0xB8
    ni = np.zeros((P, P), np.uint8)
    ni[np.arange(P), np.arange(P)] = 0xB8
    wvp[:, WI + P : WT] = ni.view(np.uint16).view(bf)

    bq = chunk_vec(b_qkv[q_idx] * s1)
    bk = chunk_vec(b_qkv[k_idx] * s1)
    # v-bias folded into the proj bias: proj(a + bv) = proj(a) + w_proj @ bv
    bp_eff = b_proj + w_proj @ b_qkv[v_idx]
    bp = chunk_vec(bp_eff)

    cidx = np.arange(C)
    sel8 = np.zeros((P, KC, GC), f)
    sel8[cidx % P, cidx // P, (cidx % P) // 16] = 1.0 / 16.0
    selt8 = np.zeros((GC, KC, P), f)
    selt8[(cidx % P) // 16, cidx // P, cidx % P] = 1.0
    sel32 = np.zeros((P, KC, GROUPS), f)
    sel32[cidx % P, cidx // P, cidx // 16] = 1.0 / 16.0
    selt32 = np.zeros((GROUPS, KC, P), f)
    selt32[cidx // 16, cidx // P, cidx % P] = 1.0

    pars = np.zeros((P, 448), f)
    pars[:, 0:4] = chunk_vec(gn1_scale)
    pars[:, 4:8] = chunk_vec(gn1_bias)
    pars[:, 8:12] = bq
    pars[:, 12:16] = bk
    pars[:, 16:20] = bp
    pars[:, 20:24] = chunk_vec(gn2_scale)
    pars[:, 24:28] = chunk_vec(gn2_bias)
    pars[:, 28:60] = sel8.reshape(P, -1)
    pars[:, 60:188] = sel32.reshape(P, -1)
    pars[:, 444] = 1.0
    parsg = np.zeros((GROUPS, 1544), f)
    parsg[:, 0:512] = selt32.reshape(GROUPS, -1)
    parsg[:, 1024] = EPS
    parsg[0:GC, 1028:1540] = selt8.reshape(GC, -1)

    shared = {
        "wqk": wqk, "wvp": wvp, "pars": pars, "parsg": parsg,
    }
    in_maps = []
    for b in range(B):
        xb = np.ascontiguousarray(
            x[b].reshape(C, N).reshape(KC, P, N).transpose(1, 0, 2)
        )
        in_maps.append({"x": xb, "x8": xb.astype(bf), **shared})
    return in_maps


def _assemble(results):
    out = np.empty((B, C, H, W), np.float32)
    for b in range(B):
        ob = np.asarray(results[b]["out"])  # [P, KC, N]
        out[b] = ob.transpose(1, 0, 2).reshape(C, N).reshape(C, H, W)
    return out


def get_nc():
    if "nc" not in _CACHE:
        _CACHE["nc"] = _build_nc()
    return _CACHE["nc"]


def kernel(x, gn1_scale, gn1_bias, w_qkv, b_qkv, w_proj, b_proj, gn2_scale, gn2_bias):
    from concourse.bass_utils import run_bass_kernel_spmd

    nc = get_nc()
    in_maps = _host_prep(
        x, gn1_scale, gn1_bias, w_qkv, b_qkv, w_proj, b_proj, gn2_scale, gn2_bias
    )
    res = run_bass_kernel_spmd(nc, in_maps, core_ids=list(range(B)))
    return _assemble(res.results)
